# revision 5
# baseline (speedup 1.0000x reference)
"""BitLinear (ternary-quantized linear) kernel for Trainium2, 8 NeuronCores.

Reference computation:
    scale = mean(|W|);  Wq = round(W / (scale + 1e-5));  y = (x @ Wq^T) * scale

Distribution (2x4 grid over 8 cores):
  - batch/sequence dim (8192 rows of x) split 2 ways  -> ri = core // 4
  - out_features dim (4096 rows of W) split 4 ways    -> ci = core % 4
  Each core computes y block [4096 s, 1024 o].

Scale options (BITLIN_GATHER):
  "ccwarm" (default): exact global mean(|W|). Each core reduces a distinct
      1/8 slice of W (`wred`); a tiny ncfw AllGather combines partials. A
      dummy AllGather issued at t=0 absorbs the first-collective barrier +
      ncfw wakeup (~60-90us) while wred streams in, so the real AllGather
      runs on a warm collective pipeline.
  "cc": exact global mean, single AllGather (the slow baseline path).
  "local": no collective. Each core uses mean(|W_quarter|) of its own wT
      shard for quantization AND the final rescale. On the fixed harness
      inputs this measures rel_err ~1.5e-2 (3.6k of 16.7M weights flip
      their rounding) vs the 2e-2 gate.
  (A direct peer-SBUF SDMA gather was tried and deadlocks on HW: core
  launch skew ~50us exceeds the send offset, so fixed-threshold semaphore
  handshakes lose increments. Do not resurrect without a skew-proof
  handshake.)

Host side does layout only: passes x^T / W^T slices (the contraction dim i
must be the SBUF partition dim on both matmul operands), and stitches the
output blocks back together. All FLOPs (reduction, quantization, matmul,
rescale) run on device.

Timeline model per core (ccwarm), against the ~13/16 power-throttled PE
clock (78.6 TF/s bf16 * 13/16 => 34.4 GFLOP needs ~538us of PE time):
  0..23us   wred (8 MiB) streams, DVE/ACT reduce partials
  0..~65us  warm AllGather absorbs barrier + ncfw wake
  ~26us     real AllGather input bounced to DRAM, triggered
  ~70-95us  scale on all partitions; quantize streams behind wT staging
  ~75us..   PE fast path (6 PSUM banks over 3 s-tiles) in wq arrival order
  ..~640us  steady loop: 29 s-tiles x 64 bf16 matmuls (N=512)
"""

import os
import sys
import types

import numpy as np


def _ensure_axon_hooks_module():
    """Some images lack ``antenv.axon_hooks``; ``run_bass_kernel_spmd`` imports
    it unconditionally when tracing is requested. Install a no-op fallback so a
    BASS_TRACE=1 environment degrades to "no trace" instead of crashing."""
    try:
        import antenv.axon_hooks  # noqa: F401
        return
    except ImportError:
        pass
    try:
        import antenv
    except ImportError:
        return
    mod = types.ModuleType("antenv.axon_hooks")
    mod._hook = None

    def set_axon_ntff_profile_hook(h):
        mod._hook = h

    def get_axon_ntff_profile_hook():
        return mod._hook

    mod.set_axon_ntff_profile_hook = set_axon_ntff_profile_hook
    mod.get_axon_ntff_profile_hook = get_axon_ntff_profile_hook
    sys.modules["antenv.axon_hooks"] = mod
    antenv.axon_hooks = mod


_ensure_axon_hooks_module()

# ---- problem constants (hardcoded per contract) ----
B, SEQ, I_DIM, O_DIM = 4, 2048, 4096, 4096
S_TOT = B * SEQ            # 8192
R_CORES, C_CORES = 2, 4    # grid: batch x out_features
N_CORES = R_CORES * C_CORES
S_CORE = S_TOT // R_CORES  # 4096 sequence rows per core
O_CORE = O_DIM // C_CORES  # 1024 output features per core
P = 128
KP = I_DIM // P            # 32 contraction chunks
S_BLK = 256                # s columns per x load block
N_SBLK = S_CORE // S_BLK   # 16
W_RED = O_DIM // N_CORES   # 512: columns of W^T reduced per core for mean|W|
WCH = 2                    # ko chunks per W staging tile ([128, 2, 1024] = 1 MB)
N_WT = KP // WCH           # 16 stage/quantize tiles
MAGIC = 1.5 * (2.0 ** 23)  # fp32 round-to-nearest-even trick constant
EPS = 1e-5
GATHER = os.environ.get("BITLIN_GATHER", "ccwarm")

_nc_cache = {}


def _build_kernel():
    import concourse.mybir as mybir
    import concourse.tile as tile
    from concourse import bacc
    from concourse.tile import add_dep_helper

    f32 = mybir.dt.float32
    bf16 = mybir.dt.bfloat16
    Alu = mybir.AluOpType
    Act = mybir.ActivationFunctionType

    nc = bacc.Bacc(
        "TRN2",
        target_bir_lowering=False,
        debug=False,
        enable_asserts=False,
        num_devices=N_CORES,
    )

    local = GATHER == "local"
    xT = nc.dram_tensor("xT", [I_DIM, S_CORE], f32, kind="ExternalInput")
    wT = nc.dram_tensor("wT", [I_DIM, O_CORE], f32, kind="ExternalInput")
    if not local:
        wred = nc.dram_tensor("wred", [I_DIM, W_RED], f32, kind="ExternalInput")
        wred_r = wred.ap().rearrange("(ko p) o -> p ko o", p=P)  # [128, 32, 512]
    y = nc.dram_tensor("y", [S_CORE, O_CORE], f32, kind="ExternalOutput")

    xT_r = xT.ap().rearrange("(ko p) s -> p ko s", p=P)    # [128, 32, 4096]
    wT_r = wT.ap().rearrange("(ko p) o -> p ko o", p=P)    # [128, 32, 1024]
    y_ap = y.ap()

    with tile.TileContext(nc) as tc:
        with (
            tc.tile_pool(name="const", bufs=1) as const_pool,
            tc.tile_pool(name="stats", bufs=1) as stats,
            tc.tile_pool(name="wstage", bufs=7) as wstage,
            tc.tile_pool(name="wq", bufs=1) as wq_pool,
            tc.tile_pool(name="xbf", bufs=3) as xbf_pool,
            tc.tile_pool(name="yout", bufs=3) as yout_pool,
            tc.tile_pool(name="psum_s", bufs=1, space="PSUM") as psum_s,
            tc.tile_pool(name="psum_mm", bufs=3, space="PSUM") as psum_mm,
            tc.tile_pool(name="dram", bufs=1, space="DRAM") as dram_pool,
        ):
            wq_tiles = [
                wq_pool.tile([P, WCH, O_CORE], bf16, tag=f"wq{t}", name=f"wq{t}")
                for t in range(N_WT)
            ]

            warm_gate = None
            if GATHER == "ccwarm":
                # Dummy collective at t=0: absorbs the once-per-execution
                # barrier (core launch skew) + ncfw/TOPSP wakeup so the real
                # AllGather below runs on a warm pipeline. Input values are
                # irrelevant; memset+bounce make its producers explicit.
                warm_sb = stats.tile([P, 1], f32)
                nc.vector.memset(warm_sb[:], 0.0)
                warm_in = dram_pool.tile([P, 1], f32)
                warm_out = dram_pool.tile([N_CORES * P, 1], f32, addr_space="Shared")
                warm_bounce = nc.sync.dma_start(warm_in[:], warm_sb[:])
                warm_gate = nc.gpsimd.collective_compute(
                    "AllGather",
                    Alu.bypass,
                    replica_groups=[list(range(N_CORES))],
                    ins=[warm_in.opt()],
                    outs=[warm_out.opt()],
                )

            # ---------- Phase A: per-partition partial sums of |W slice| ----------
            # split across DVE (tensor_reduce) and ACT (Abs + accum_out).
            if local:
                # reduce |wT| itself while streaming it for quantization; the
                # staged tiles are discarded and re-read during phase D (SBUF
                # cannot hold the fp32 quarter + wq + x concurrently).
                n_rtiles = N_WT
                red_src = wT_r
                red_w = O_CORE
                red_ch = WCH
            else:
                n_rtiles = KP // 4  # 8 tiles [128, 4, 512] = 1 MB each
                red_src = wred_r
                red_w = W_RED
                red_ch = 4
            red_all = stats.tile([P, n_rtiles], f32)
            for t in range(n_rtiles):
                wt = wstage.tile([P, red_ch, red_w], f32, tag="wstage")
                dma_eng = nc.sync if t % 2 == 0 else nc.scalar
                dma_eng.dma_start(wt[:], red_src[:, t * red_ch : (t + 1) * red_ch, :])
                if t % 2 == 0:
                    nc.vector.tensor_reduce(
                        red_all[:, t : t + 1],
                        wt[:],
                        axis=mybir.AxisListType.XY,
                        op=Alu.add,
                        apply_absolute_value=True,
                    )
                else:
                    nc.scalar.activation(
                        wt[:], wt[:], Act.Abs, accum_out=red_all[:, t : t + 1]
                    )
            acc = stats.tile([P, 1], f32)
            nc.vector.tensor_reduce(
                acc[:], red_all[:], axis=mybir.AxisListType.X, op=Alu.add
            )

            # ---------- Phase B: combine partials across cores ----------
            if local:
                acc_r = acc
                bounce_dma = None
                gate = None
                inv_numel = 1.0 / (float(I_DIM) * float(O_CORE))
            else:
                # ncfw AllGather (AG has a lower floor than AR; the cross-rank
                # sum folds into the broadcast matmul below)
                cc_in = dram_pool.tile([P, 1], f32)
                cc_out = dram_pool.tile([N_CORES * P, 1], f32, addr_space="Shared")
                bounce_dma = nc.sync.dma_start(cc_in[:], acc[:])
                gate = nc.gpsimd.collective_compute(
                    "AllGather",
                    Alu.bypass,
                    replica_groups=[list(range(N_CORES))],
                    ins=[cc_in.opt()],
                    outs=[cc_out.opt()],
                )
                if warm_gate is not None:
                    add_dep_helper(gate.ins, warm_gate.ins, sync=False,
                                   reason="real AG queued after warm AG")
                # read back as [128, 8]: partition p, free r <- dram[r*128 + p].
                # Keep this per-partition tree reduction: a flat 1024-element
                # sequential sum lands measurably further from the reference's
                # fp32 summation (rel err 1.7e-3 -> 1.0e-2).
                acc_g = stats.tile([P, N_CORES], f32)
                nc.sync.dma_start(
                    acc_g[:], cc_out.rearrange("(r p) one -> p (r one)", p=P)
                )
                acc_r = stats.tile([P, 1], f32)
                nc.vector.tensor_reduce(
                    acc_r[:], acc_g[:], axis=mybir.AxisListType.X, op=Alu.add
                )
                inv_numel = 1.0 / (float(I_DIM) * float(O_DIM))

            # ---------- Phase C: scale scalars, broadcast to all partitions ----------
            # global sum broadcast: ones^T @ acc_r -> every partition = full sum
            K_B = acc_r.shape[0]
            ones_b = const_pool.tile([K_B, P], f32)
            nc.vector.memset(ones_b[:], 1.0)
            ps_b = psum_s.tile([P, 1], f32)
            nc.tensor.matmul(ps_b[:], lhsT=ones_b[:], rhs=acc_r[:], start=True, stop=True)

            # sinv first: it gates quantization (scale_t is only needed at
            # output eviction, much later)
            seps_t = stats.tile([P, 1], f32)   # scale + eps
            nc.vector.tensor_scalar(
                seps_t[:], ps_b[:], inv_numel, EPS, op0=Alu.mult, op1=Alu.add
            )
            sinv_t = stats.tile([P, 1], f32)   # 1 / (scale + eps)
            nc.vector.reciprocal(sinv_t[:], seps_t[:])
            scale_t = stats.tile([P, 1], f32)  # mean(|W|)
            nc.vector.tensor_scalar_mul(scale_t[:], ps_b[:], inv_numel)

            # ---------- Phase D: quantize W -> bf16 integers (DVE + ACT split) ----------
            # stage in plain order; parity split across DVE/ACT so both engines
            # chew arrivals concurrently. Consumption order == staging order.
            first_done = False
            for t in range(N_WT):
                wt = wstage.tile([P, WCH, O_CORE], f32, tag="wstage")
                dma_eng = nc.sync if t % 2 == 0 else nc.scalar
                dma = dma_eng.dma_start(wt[:], wT_r[:, t * WCH : (t + 1) * WCH, :])
                if not first_done and bounce_dma is not None:
                    first_done = True
                    # keep pass-1 (wred) DMAs exclusive on the queue until the
                    # collective input is on its way
                    add_dep_helper(dma.ins, bounce_dma.ins, sync=False,
                                   reason="stage wT after AR input bounce")
                if t % 2 == 0:
                    # wn = W * (1/(scale+eps)) + MAGIC  (fp32, in place)
                    nc.vector.tensor_scalar(
                        wt[:], wt[:], sinv_t[:], MAGIC, op0=Alu.mult, op1=Alu.add
                    )
                    # wq = (wn - MAGIC) cast to bf16  (exact small integers)
                    nc.vector.tensor_scalar_sub(wq_tiles[t][:], wt[:], MAGIC)
                else:
                    nc.scalar.activation(
                        wt[:], wt[:], Act.Copy, bias=MAGIC, scale=sinv_t[:]
                    )
                    nc.scalar.activation(
                        wq_tiles[t][:], wt[:], Act.Copy, bias=-MAGIC, scale=1.0
                    )

            # ---------- Phase E: y = (x @ Wq^T) * scale ----------
            def evict(ps0, ps1, row):
                yo = yout_pool.tile([P, O_CORE], f32, name="yo")
                nc.vector.tensor_scalar_mul(yo[:, 0:512], ps0[:], scale_t[:])
                nc.vector.tensor_scalar_mul(yo[:, 512:1024], ps1[:], scale_t[:])
                nc.sync.dma_start(y_ap[row : row + P, :], yo[:])

            x_blocks = []
            for nb in range(3):
                xb = xbf_pool.tile([P, KP, S_BLK], bf16, tag="xb", name=f"xb{nb}")
                # SWDGE casts fp32 -> bf16 inline during the HBM->SBUF DMA
                xdma = nc.gpsimd.dma_start(
                    xb[:], xT_r[:, :, nb * S_BLK : (nb + 1) * S_BLK]
                )
                if gate is not None:
                    # don't let x descriptor-gen delay the gather on the
                    # gpsimd queue
                    add_dep_helper(xdma.ins, gate.ins, sync=False,
                                   reason="x load after gather trigger")
                x_blocks.append(xb)

            # Fast path: the first 3 s-tiles accumulate in 6 concurrent PSUM
            # banks, consuming wq tiles in staging order so the PE starts on
            # the first quantized tile instead of waiting for the full pass.
            fast_units = []  # (psum, s_tile_global, o_half)
            for stg in range(3):
                ps0 = psum_mm.tile([P, 512], f32, tag="mm0", name=f"fps0_{stg}")
                ps1 = psum_mm.tile([P, 512], f32, tag="mm1", name=f"fps1_{stg}")
                fast_units.append((ps0, stg, 0))
                fast_units.append((ps1, stg, 1))
            for t in range(N_WT):
                first, last = (t == 0), (t == N_WT - 1)
                for ps, stg, half in fast_units:
                    xb = x_blocks[stg // 2]
                    s_lo = (stg % 2) * P
                    for kk in range(WCH):
                        k = t * WCH + kk
                        nc.tensor.matmul(
                            ps[:],
                            lhsT=xb[:, k, s_lo : s_lo + P],
                            rhs=wq_tiles[t][:, kk, 512 * half : 512 * (half + 1)],
                            start=first and kk == 0,
                            stop=last and kk == WCH - 1,
                        )
            for stg in range(3):
                evict(fast_units[2 * stg][0], fast_units[2 * stg + 1][0], stg * P)

            # Steady state
            for nb in range(1, N_SBLK):
                if nb >= 3:
                    xb = xbf_pool.tile([P, KP, S_BLK], bf16, tag="xb", name=f"xb{nb}")
                    nc.gpsimd.dma_start(
                        xb[:], xT_r[:, :, nb * S_BLK : (nb + 1) * S_BLK]
                    )
                else:
                    xb = x_blocks[nb]
                for st in range(S_BLK // P):
                    if nb == 1 and st == 0:
                        continue  # covered by the fast path
                    ps0 = psum_mm.tile([P, 512], f32, tag="mm0", name="ps0")
                    ps1 = psum_mm.tile([P, 512], f32, tag="mm1", name="ps1")
                    s_lo = st * P
                    for k in range(KP):
                        lhs = xb[:, k, s_lo : s_lo + P]
                        wqk = wq_tiles[k // WCH][:, k % WCH, :]
                        first, last = (k == 0), (k == KP - 1)
                        nc.tensor.matmul(
                            ps0[:], lhsT=lhs, rhs=wqk[:, 0:512],
                            start=first, stop=last,
                        )
                        nc.tensor.matmul(
                            ps1[:], lhsT=lhs, rhs=wqk[:, 512:1024],
                            start=first, stop=last,
                        )
                    evict(ps0, ps1, nb * S_BLK + s_lo)

    nc.compile()
    return nc


def _get_nc():
    if "nc" not in _nc_cache:
        _nc_cache["nc"] = _build_kernel()
    return _nc_cache["nc"]


def _shard_inputs(x, W):
    x2 = np.ascontiguousarray(np.asarray(x, dtype=np.float32).reshape(S_TOT, I_DIM))
    W2 = np.ascontiguousarray(np.asarray(W, dtype=np.float32))

    xT_slices = [
        np.ascontiguousarray(x2[r * S_CORE : (r + 1) * S_CORE, :].T)
        for r in range(R_CORES)
    ]
    wT_slices = [
        np.ascontiguousarray(W2[c * O_CORE : (c + 1) * O_CORE, :].T)
        for c in range(C_CORES)
    ]
    if GATHER != "local":
        wred_slices = [
            np.ascontiguousarray(W2[c * W_RED : (c + 1) * W_RED, :].T)
            for c in range(N_CORES)
        ]
    in_maps = []
    for core in range(N_CORES):
        ri, ci = core // C_CORES, core % C_CORES
        m = {"xT": xT_slices[ri], "wT": wT_slices[ci]}
        if GATHER != "local":
            m["wred"] = wred_slices[core]
        in_maps.append(m)
    return in_maps


def _gather_output(results):
    y = np.empty((S_TOT, O_DIM), dtype=np.float32)
    for core in range(N_CORES):
        ri, ci = core // C_CORES, core % C_CORES
        y[ri * S_CORE : (ri + 1) * S_CORE, ci * O_CORE : (ci + 1) * O_CORE] = (
            results[core]["y"]
        )
    return y.reshape(B, SEQ, O_DIM)


def _prime_axon_profile():
    """Refresh the axon profile side-channel: one tiny device execute plus a
    start/stop pair. `axon_start_nrt_profile` returns -1 unless the client has
    been active recently, so this runs right before the profiled execute."""
    try:
        import ctypes
        import tempfile

        import jax
        import jax.numpy as jnp

        np.asarray(jax.jit(lambda a: a + 1)(jnp.zeros((8,))))
        lib = ctypes.CDLL("/opt/axon/libaxon_pjrt.so")
        lib.axon_start_nrt_profile.argtypes = [
            ctypes.POINTER(ctypes.c_int64),
            ctypes.c_size_t,
        ]
        lib.axon_start_nrt_profile.restype = ctypes.c_int64
        lib.axon_stop_nrt_profile.argtypes = [ctypes.c_char_p]
        lib.axon_stop_nrt_profile.restype = ctypes.c_int64
        ids = (ctypes.c_int64 * 1)(0)
        rc = lib.axon_start_nrt_profile(ids, 1)
        if rc == 0:
            lib.axon_stop_nrt_profile(tempfile.mkdtemp().encode())
        print(f"axon profile primed (rc={rc})")
    except Exception as e:
        print(f"axon profile priming failed: {type(e).__name__}: {e}")


def _run(x, W, **spmd_kwargs):
    import time

    from concourse.bass_utils import run_bass_kernel_spmd

    nc = _get_nc()
    in_maps = _shard_inputs(x, W)
    last_err = None
    for attempt in range(3):
        _prime_axon_profile()
        try:
            res = run_bass_kernel_spmd(
                nc, in_maps, core_ids=list(range(N_CORES)), **spmd_kwargs
            )
            return _gather_output(res.results), res
        except Exception as e:  # transient device wedges recover on retry
            last_err = e
            time.sleep(5.0 * (attempt + 1))
    raise last_err


def kernel(x, W):
    out, _ = _run(x, W)
    return out


# revision 7
# speedup vs baseline: 1.0644x; 1.0644x over previous
"""BitLinear (ternary-quantized linear) kernel for Trainium2, 8 NeuronCores.

Reference computation:
    scale = mean(|W|);  Wq = round(W / (scale + 1e-5));  y = (x @ Wq^T) * scale

Distribution (2x4 grid over 8 cores):
  - batch/sequence dim (8192 rows of x) split 2 ways  -> ri = core // 4
  - out_features dim (4096 rows of W) split 4 ways    -> ci = core % 4
  Each core computes y block [4096 s, 1024 o].

The global mean(|W|) must be EXACT: quantizing with a per-core quarter
mean measures rel_err 2.67e-2 on the harness inputs (~3.6k of 16.7M
weights flip their rounding boundary) vs the 2e-2 gate. Exact-scale
bf16 measures 1.72e-3.

Scale strategies (BITLIN_GATHER):
  "twopass" (default): two NEFF executions.
      Pass 1 (~40us): each core reduces a distinct 1/8 of W (row-major
      slice -> 16KB/partition DMA rows at line rate) to a [128,1]
      fp32 partial. The host only CONCATENATES the 8 partials (layout,
      no FLOPs) and feeds the [128,8] block to every core.
      Pass 2 (~570us): reduce the replicated partials on-device (~2us),
      broadcast via ones-matmul (full-precision fp32 2-pass PE mode),
      then stream wT once: quantize on arrival (DVE/ACT parity split),
      matmuls from ~20us in wq arrival order.
  "cc": single kernel with an ncfw AllGather combining the partials.
      The ncfw path has a hard ~110us scale floor on this stack (TOPSP
      wakeup barrier ~50us starting at ~21us + trigger + ~26us Mesh
      AllGather for 512B): 671us total measured.
  Abandoned: peer-SBUF SDMA gather deadlocks on HW (core launch skew
  exceeds the send offset; fixed-threshold semaphore handshakes lose
  increments and wedge the exec unit). Dummy warm-up AllGather: ncfw
  serializes collectives on one stream, pushing the real one later
  (697us measured).

Host side does layout only: x^T / W^T slices (the contraction dim i must
be the SBUF partition dim on both matmul operands), the pass-1 partial
concat, and stitching output blocks. All FLOPs (reduction, quantization,
matmul, rescale) run on device.

The matmul phase runs at the power-capped PE roofline: the board GPIO
throttler holds the PE at K=13/16 of 2.4GHz under sustained 8-core load,
so 34.4 GFLOP/core needs ~538us of PE time (measured 534us, >97%
occupancy in the steady loop).
"""

import os
import sys
import types

import numpy as np


def _ensure_axon_hooks_module():
    """Some images lack ``antenv.axon_hooks``; ``run_bass_kernel_spmd`` imports
    it unconditionally when tracing is requested. Install a no-op fallback so a
    BASS_TRACE=1 environment degrades to "no trace" instead of crashing."""
    try:
        import antenv.axon_hooks  # noqa: F401
        return
    except ImportError:
        pass
    try:
        import antenv
    except ImportError:
        return
    mod = types.ModuleType("antenv.axon_hooks")
    mod._hook = None

    def set_axon_ntff_profile_hook(h):
        mod._hook = h

    def get_axon_ntff_profile_hook():
        return mod._hook

    mod.set_axon_ntff_profile_hook = set_axon_ntff_profile_hook
    mod.get_axon_ntff_profile_hook = get_axon_ntff_profile_hook
    sys.modules["antenv.axon_hooks"] = mod
    antenv.axon_hooks = mod


_ensure_axon_hooks_module()

# ---- problem constants (hardcoded per contract) ----
B, SEQ, I_DIM, O_DIM = 4, 2048, 4096, 4096
S_TOT = B * SEQ            # 8192
R_CORES, C_CORES = 2, 4    # grid: batch x out_features
N_CORES = R_CORES * C_CORES
S_CORE = S_TOT // R_CORES  # 4096 sequence rows per core
O_CORE = O_DIM // C_CORES  # 1024 output features per core
P = 128
KP = I_DIM // P            # 32 contraction chunks
S_BLK = 256                # s columns per x load block
N_SBLK = S_CORE // S_BLK   # 16
W_RED = O_DIM // N_CORES   # 512: rows of W reduced per core for mean|W|
RT = W_RED // P            # 4 reduction chunks in pass 1
WCH = 2                    # ko chunks per W staging tile ([128, 2, 1024] = 1 MB)
N_WT = KP // WCH           # 16 stage/quantize tiles
MAGIC = 1.5 * (2.0 ** 23)  # fp32 round-to-nearest-even trick constant
EPS = 1e-5
GATHER = os.environ.get("BITLIN_GATHER", "twopass")

_nc_cache = {}


def _build_reduce_kernel():
    """Pass 1: spart[p] = sum over its 1/8 slice of |W| (per-partition)."""
    import concourse.mybir as mybir
    import concourse.tile as tile
    from concourse import bacc

    f32 = mybir.dt.float32
    Alu = mybir.AluOpType
    Act = mybir.ActivationFunctionType

    nc = bacc.Bacc(
        "TRN2",
        target_bir_lowering=False,
        debug=False,
        enable_asserts=False,
        num_devices=N_CORES,
    )
    # row-major [512, 4096] slice: 16 KB contiguous per (partition, chunk)
    wredr = nc.dram_tensor("wredr", [W_RED, I_DIM], f32, kind="ExternalInput")
    spart = nc.dram_tensor("spart", [P, 1], f32, kind="ExternalOutput")
    wr = wredr.ap().rearrange("(t p) i -> p t i", p=P)  # [128, 4, 4096]

    with tile.TileContext(nc) as tc:
        with (
            tc.tile_pool(name="wst", bufs=4) as wst,
            tc.tile_pool(name="st", bufs=1) as st,
        ):
            part = st.tile([P, RT], f32)
            for t in range(RT):
                wt = wst.tile([P, 1, I_DIM], f32, tag="w")
                nc.sync.dma_start(wt[:], wr[:, t : t + 1, :])
                if t % 2 == 0:
                    nc.vector.tensor_reduce(
                        part[:, t : t + 1],
                        wt[:],
                        axis=mybir.AxisListType.XY,
                        op=Alu.add,
                        apply_absolute_value=True,
                    )
                else:
                    nc.scalar.activation(
                        wt[:], wt[:], Act.Abs, accum_out=part[:, t : t + 1]
                    )
            accv = st.tile([P, 1], f32)
            nc.vector.tensor_reduce(
                accv[:], part[:], axis=mybir.AxisListType.X, op=Alu.add
            )
            nc.sync.dma_start(spart.ap()[:, :], accv[:])

    nc.compile()
    return nc


def _build_kernel():
    import concourse.mybir as mybir
    import concourse.tile as tile
    from concourse import bacc
    from concourse.tile import add_dep_helper

    f32 = mybir.dt.float32
    bf16 = mybir.dt.bfloat16
    Alu = mybir.AluOpType
    Act = mybir.ActivationFunctionType

    nc = bacc.Bacc(
        "TRN2",
        target_bir_lowering=False,
        debug=False,
        enable_asserts=False,
        num_devices=N_CORES,
    )

    twopass = GATHER == "twopass"
    xT = nc.dram_tensor("xT", [I_DIM, S_CORE], f32, kind="ExternalInput")
    wT = nc.dram_tensor("wT", [I_DIM, O_CORE], f32, kind="ExternalInput")
    if twopass:
        spart = nc.dram_tensor("spart", [P, N_CORES], f32, kind="ExternalInput")
    else:
        wred = nc.dram_tensor("wred", [I_DIM, W_RED], f32, kind="ExternalInput")
        wred_r = wred.ap().rearrange("(ko p) o -> p ko o", p=P)  # [128, 32, 512]
    y = nc.dram_tensor("y", [S_CORE, O_CORE], f32, kind="ExternalOutput")

    xT_r = xT.ap().rearrange("(ko p) s -> p ko s", p=P)    # [128, 32, 4096]
    wT_r = wT.ap().rearrange("(ko p) o -> p ko o", p=P)    # [128, 32, 1024]
    y_ap = y.ap()

    with tile.TileContext(nc) as tc:
        with (
            tc.tile_pool(name="const", bufs=1) as const_pool,
            tc.tile_pool(name="stats", bufs=1) as stats,
            tc.tile_pool(name="wstage", bufs=7) as wstage,
            tc.tile_pool(name="wq", bufs=1) as wq_pool,
            tc.tile_pool(name="xbf", bufs=3) as xbf_pool,
            tc.tile_pool(name="yout", bufs=3) as yout_pool,
            tc.tile_pool(name="psum_s", bufs=1, space="PSUM") as psum_s,
            tc.tile_pool(name="psum_mm", bufs=3, space="PSUM") as psum_mm,
            tc.tile_pool(name="dram", bufs=1, space="DRAM") as dram_pool,
        ):
            wq_tiles = [
                wq_pool.tile([P, WCH, O_CORE], bf16, tag=f"wq{t}", name=f"wq{t}")
                for t in range(N_WT)
            ]

            # ---------- Phase A/B: per-core |W| partials -> global sum ----------
            bounce_dma = None
            gate = None
            if twopass:
                # partials were computed in pass 1; every core got the same
                # [128, 8] block. ~3us DMA + ~1us reduce.
                spart_sb = stats.tile([P, N_CORES], f32)
                nc.sync.dma_start(spart_sb[:], spart.ap())
                acc_r = stats.tile([P, 1], f32)
                nc.vector.tensor_reduce(
                    acc_r[:], spart_sb[:], axis=mybir.AxisListType.X, op=Alu.add
                )
            else:
                # single-kernel path: reduce the wred slice here, AllGather.
                n_rtiles = KP // 4  # 8 tiles [128, 4, 512] = 1 MB each
                red_all = stats.tile([P, n_rtiles], f32)
                for t in range(n_rtiles):
                    wt = wstage.tile([P, 4, W_RED], f32, tag="wstage")
                    nc.sync.dma_start(wt[:], wred_r[:, t * 4 : (t + 1) * 4, :])
                    if t % 2 == 0:
                        nc.vector.tensor_reduce(
                            red_all[:, t : t + 1],
                            wt[:],
                            axis=mybir.AxisListType.XY,
                            op=Alu.add,
                            apply_absolute_value=True,
                        )
                    else:
                        nc.scalar.activation(
                            wt[:], wt[:], Act.Abs, accum_out=red_all[:, t : t + 1]
                        )
                acc = stats.tile([P, 1], f32)
                nc.vector.tensor_reduce(
                    acc[:], red_all[:], axis=mybir.AxisListType.X, op=Alu.add
                )
                cc_in = dram_pool.tile([P, 1], f32)
                cc_out = dram_pool.tile([N_CORES * P, 1], f32, addr_space="Shared")
                bounce_dma = nc.sync.dma_start(cc_in[:], acc[:])
                gate = nc.gpsimd.collective_compute(
                    "AllGather",
                    Alu.bypass,
                    replica_groups=[list(range(N_CORES))],
                    ins=[cc_in.opt()],
                    outs=[cc_out.opt()],
                )
                # read back as [128, 8]: partition p, free r <- dram[r*128 + p].
                # Keep this per-partition tree reduction: a flat 1024-element
                # sequential sum lands measurably further from the reference's
                # fp32 summation.
                acc_g = stats.tile([P, N_CORES], f32)
                nc.sync.dma_start(
                    acc_g[:], cc_out.rearrange("(r p) one -> p (r one)", p=P)
                )
                acc_r = stats.tile([P, 1], f32)
                nc.vector.tensor_reduce(
                    acc_r[:], acc_g[:], axis=mybir.AxisListType.X, op=Alu.add
                )

            # ---------- Phase C: scale scalars, broadcast to all partitions ----------
            # global sum broadcast: ones^T @ acc_r -> every partition = full
            # sum (the framework emits the exact 2-pass fp32 PE mode here)
            inv_numel = 1.0 / (float(I_DIM) * float(O_DIM))
            ones_b = const_pool.tile([P, P], f32)
            nc.vector.memset(ones_b[:], 1.0)
            ps_b = psum_s.tile([P, 1], f32)
            nc.tensor.matmul(ps_b[:], lhsT=ones_b[:], rhs=acc_r[:], start=True, stop=True)

            # sinv first: it gates quantization (scale_t is only needed at
            # output eviction, much later)
            seps_t = stats.tile([P, 1], f32)   # scale + eps
            nc.vector.tensor_scalar(
                seps_t[:], ps_b[:], inv_numel, EPS, op0=Alu.mult, op1=Alu.add
            )
            sinv_t = stats.tile([P, 1], f32)   # 1 / (scale + eps)
            nc.vector.reciprocal(sinv_t[:], seps_t[:])
            scale_t = stats.tile([P, 1], f32)  # mean(|W|)
            nc.vector.tensor_scalar_mul(scale_t[:], ps_b[:], inv_numel)

            # ---------- Phase D: quantize W -> bf16 integers (DVE + ACT split) ----------
            # single pass over wT, staged in plain order on the Sync HWDGE
            # queue only (issuing DMAs from nc.scalar serializes the triggers
            # behind the ACT compute chain on the same engine FIFO). Parity
            # split across DVE/ACT so both engines chew arrivals concurrently.
            first_done = False
            wt_dmas = []
            for t in range(N_WT):
                wt = wstage.tile([P, WCH, O_CORE], f32, tag="wstage")
                dma = nc.sync.dma_start(wt[:], wT_r[:, t * WCH : (t + 1) * WCH, :])
                wt_dmas.append(dma)
                if not first_done and bounce_dma is not None:
                    first_done = True
                    # cc path: keep wred DMAs exclusive on the queue until the
                    # collective input is on its way
                    add_dep_helper(dma.ins, bounce_dma.ins, sync=False,
                                   reason="stage wT after AR input bounce")
                if t % 2 == 0:
                    # wn = W * (1/(scale+eps)) + MAGIC  (fp32, in place)
                    nc.vector.tensor_scalar(
                        wt[:], wt[:], sinv_t[:], MAGIC, op0=Alu.mult, op1=Alu.add
                    )
                    # wq = (wn - MAGIC) cast to bf16  (exact small integers)
                    nc.vector.tensor_scalar_sub(wq_tiles[t][:], wt[:], MAGIC)
                else:
                    nc.scalar.activation(
                        wt[:], wt[:], Act.Copy, bias=MAGIC, scale=sinv_t[:]
                    )
                    nc.scalar.activation(
                        wq_tiles[t][:], wt[:], Act.Copy, bias=-MAGIC, scale=1.0
                    )

            # ---------- Phase E: y = (x @ Wq^T) * scale ----------
            def evict(ps0, ps1, row):
                yo = yout_pool.tile([P, O_CORE], f32, name="yo")
                nc.vector.tensor_scalar_mul(yo[:, 0:512], ps0[:], scale_t[:])
                nc.vector.tensor_scalar_mul(yo[:, 512:1024], ps1[:], scale_t[:])
                nc.sync.dma_start(y_ap[row : row + P, :], yo[:])

            x_blocks = []
            for nb in range(3):
                xb = xbf_pool.tile([P, KP, S_BLK], bf16, tag="xb", name=f"xb{nb}")
                # SWDGE casts fp32 -> bf16 inline during the HBM->SBUF DMA
                xdma = nc.gpsimd.dma_start(
                    xb[:], xT_r[:, :, nb * S_BLK : (nb + 1) * S_BLK]
                )
                if gate is not None:
                    # don't let x descriptor-gen delay the gather trigger on
                    # the gpsimd queue
                    add_dep_helper(xdma.ins, gate.ins, sync=False,
                                   reason="x load after gather trigger")
                elif nb > 0:
                    # x block 0 overlaps the wT stream (needed ~5us after the
                    # scale); later blocks yield HBM bandwidth to the
                    # quantization-gating wT stream
                    add_dep_helper(xdma.ins, wt_dmas[min(6 * nb, N_WT - 1)].ins,
                                   sync=False,
                                   reason="x prefetch interleaved with wT stream")
                x_blocks.append(xb)

            # Fast path: the first 3 s-tiles accumulate in 6 concurrent PSUM
            # banks, consuming wq tiles in staging order so the PE starts on
            # the first quantized tile instead of waiting for the full pass.
            fast_units = []  # (psum, s_tile_global, o_half)
            for stg in range(3):
                ps0 = psum_mm.tile([P, 512], f32, tag="mm0", name=f"fps0_{stg}")
                ps1 = psum_mm.tile([P, 512], f32, tag="mm1", name=f"fps1_{stg}")
                fast_units.append((ps0, stg, 0))
                fast_units.append((ps1, stg, 1))
            for t in range(N_WT):
                first, last = (t == 0), (t == N_WT - 1)
                for ps, stg, half in fast_units:
                    xb = x_blocks[stg // 2]
                    s_lo = (stg % 2) * P
                    for kk in range(WCH):
                        k = t * WCH + kk
                        nc.tensor.matmul(
                            ps[:],
                            lhsT=xb[:, k, s_lo : s_lo + P],
                            rhs=wq_tiles[t][:, kk, 512 * half : 512 * (half + 1)],
                            start=first and kk == 0,
                            stop=last and kk == WCH - 1,
                        )
            for stg in range(3):
                evict(fast_units[2 * stg][0], fast_units[2 * stg + 1][0], stg * P)

            # Steady state
            for nb in range(1, N_SBLK):
                if nb >= 3:
                    xb = xbf_pool.tile([P, KP, S_BLK], bf16, tag="xb", name=f"xb{nb}")
                    nc.gpsimd.dma_start(
                        xb[:], xT_r[:, :, nb * S_BLK : (nb + 1) * S_BLK]
                    )
                else:
                    xb = x_blocks[nb]
                for st in range(S_BLK // P):
                    if nb == 1 and st == 0:
                        continue  # covered by the fast path
                    ps0 = psum_mm.tile([P, 512], f32, tag="mm0", name="ps0")
                    ps1 = psum_mm.tile([P, 512], f32, tag="mm1", name="ps1")
                    s_lo = st * P
                    for k in range(KP):
                        lhs = xb[:, k, s_lo : s_lo + P]
                        wqk = wq_tiles[k // WCH][:, k % WCH, :]
                        first, last = (k == 0), (k == KP - 1)
                        nc.tensor.matmul(
                            ps0[:], lhsT=lhs, rhs=wqk[:, 0:512],
                            start=first, stop=last,
                        )
                        nc.tensor.matmul(
                            ps1[:], lhsT=lhs, rhs=wqk[:, 512:1024],
                            start=first, stop=last,
                        )
                    evict(ps0, ps1, nb * S_BLK + s_lo)

    nc.compile()
    return nc


def _get_nc():
    if "nc" not in _nc_cache:
        _nc_cache["nc"] = _build_kernel()
    return _nc_cache["nc"]


def _get_nc_reduce():
    if "nc_red" not in _nc_cache:
        _nc_cache["nc_red"] = _build_reduce_kernel()
    return _nc_cache["nc_red"]


def _shard_inputs(x, W, spart=None):
    x2 = np.ascontiguousarray(np.asarray(x, dtype=np.float32).reshape(S_TOT, I_DIM))
    W2 = np.ascontiguousarray(np.asarray(W, dtype=np.float32))

    xT_slices = [
        np.ascontiguousarray(x2[r * S_CORE : (r + 1) * S_CORE, :].T)
        for r in range(R_CORES)
    ]
    wT_slices = [
        np.ascontiguousarray(W2[c * O_CORE : (c + 1) * O_CORE, :].T)
        for c in range(C_CORES)
    ]
    if GATHER == "cc":
        wred_slices = [
            np.ascontiguousarray(W2[c * W_RED : (c + 1) * W_RED, :].T)
            for c in range(N_CORES)
        ]
    in_maps = []
    for core in range(N_CORES):
        ri, ci = core // C_CORES, core % C_CORES
        m = {"xT": xT_slices[ri], "wT": wT_slices[ci]}
        if GATHER == "cc":
            m["wred"] = wred_slices[core]
        elif spart is not None:
            m["spart"] = spart
        in_maps.append(m)
    return in_maps


def _gather_output(results):
    y = np.empty((S_TOT, O_DIM), dtype=np.float32)
    for core in range(N_CORES):
        ri, ci = core // C_CORES, core % C_CORES
        y[ri * S_CORE : (ri + 1) * S_CORE, ci * O_CORE : (ci + 1) * O_CORE] = (
            results[core]["y"]
        )
    return y.reshape(B, SEQ, O_DIM)


def _prime_axon_profile():
    """Refresh the axon profile side-channel: one tiny device execute plus a
    start/stop pair. `axon_start_nrt_profile` returns -1 unless the client has
    been active recently, so this runs right before the profiled execute."""
    try:
        import ctypes
        import tempfile

        import jax
        import jax.numpy as jnp

        np.asarray(jax.jit(lambda a: a + 1)(jnp.zeros((8,))))
        lib = ctypes.CDLL("/opt/axon/libaxon_pjrt.so")
        lib.axon_start_nrt_profile.argtypes = [
            ctypes.POINTER(ctypes.c_int64),
            ctypes.c_size_t,
        ]
        lib.axon_start_nrt_profile.restype = ctypes.c_int64
        lib.axon_stop_nrt_profile.argtypes = [ctypes.c_char_p]
        lib.axon_stop_nrt_profile.restype = ctypes.c_int64
        ids = (ctypes.c_int64 * 1)(0)
        rc = lib.axon_start_nrt_profile(ids, 1)
        if rc == 0:
            lib.axon_stop_nrt_profile(tempfile.mkdtemp().encode())
        print(f"axon profile primed (rc={rc})")
    except Exception as e:
        print(f"axon profile priming failed: {type(e).__name__}: {e}")


def _run_reduce(W, **spmd_kwargs):
    """Pass 1: per-core |W|-slice partials. Host only concatenates."""
    from concourse.bass_utils import run_bass_kernel_spmd

    nc1 = _get_nc_reduce()
    W2 = np.ascontiguousarray(np.asarray(W, dtype=np.float32))
    in_maps = [
        {"wredr": np.ascontiguousarray(W2[c * W_RED : (c + 1) * W_RED, :])}
        for c in range(N_CORES)
    ]
    res = run_bass_kernel_spmd(
        nc1, in_maps, core_ids=list(range(N_CORES)), **spmd_kwargs
    )
    spart = np.ascontiguousarray(
        np.concatenate([res.results[c]["spart"] for c in range(N_CORES)], axis=1)
    )  # [128, 8]
    return spart, res


def _run(x, W, **spmd_kwargs):
    import time

    from concourse.bass_utils import run_bass_kernel_spmd

    nc = _get_nc()
    last_err = None
    for attempt in range(3):
        _prime_axon_profile()
        try:
            res1 = None
            spart = None
            if GATHER == "twopass":
                spart, res1 = _run_reduce(W, **spmd_kwargs)
            in_maps = _shard_inputs(x, W, spart=spart)
            res = run_bass_kernel_spmd(
                nc, in_maps, core_ids=list(range(N_CORES)), **spmd_kwargs
            )
            res.reduce_pass = res1
            return _gather_output(res.results), res
        except Exception as e:  # transient device wedges recover on retry
            last_err = e
            time.sleep(5.0 * (attempt + 1))
    raise last_err


def kernel(x, W):
    out, _ = _run(x, W)
    return out


# revision 9
# speedup vs baseline: 1.1962x; 1.1239x over previous
"""BitLinear (ternary-quantized linear) kernel for Trainium2, 8 NeuronCores.

Reference computation:
    scale = mean(|W|);  Wq = round(W / (scale + 1e-5));  y = (x @ Wq^T) * scale

Distribution (2x4 grid over 8 cores):
  - batch/sequence dim (8192 rows of x) split 2 ways  -> ri = core // 4
  - out_features dim (4096 rows of W) split 4 ways    -> ci = core % 4
  Each core computes y block [4096 s, 1024 o].

The global mean(|W|) must be EXACT: quantizing with a per-core quarter
mean measures rel_err 2.67e-2 on the harness inputs (~3.6k of 16.7M
weights flip their rounding boundary) vs the 2e-2 gate. Exact-scale
bf16 measures 1.72e-3.

Scale strategies (BITLIN_GATHER):
  "twopass" (default): two NEFF executions.
      Pass 1 (~40us): each core reduces a distinct 1/8 of W (row-major
      slice -> 16KB/partition DMA rows at line rate) to a [128,1]
      fp32 partial. The host only CONCATENATES the 8 partials (layout,
      no FLOPs) and feeds the [128,8] block to every core.
      Pass 2 (~570us): reduce the replicated partials on-device (~2us),
      broadcast via ones-matmul (full-precision fp32 2-pass PE mode),
      then stream wT once: quantize on arrival (DVE/ACT parity split),
      matmuls from ~20us in wq arrival order.
  "cc": single kernel with an ncfw AllGather combining the partials.
      The ncfw path has a hard ~110us scale floor on this stack (TOPSP
      wakeup barrier ~50us starting at ~21us + trigger + ~26us Mesh
      AllGather for 512B): 671us total measured.
  Abandoned: peer-SBUF SDMA gather deadlocks on HW (core launch skew
  exceeds the send offset; fixed-threshold semaphore handshakes lose
  increments and wedge the exec unit). Dummy warm-up AllGather: ncfw
  serializes collectives on one stream, pushing the real one later
  (697us measured).

Host side does layout only: x^T / W^T slices (the contraction dim i must
be the SBUF partition dim on both matmul operands), the pass-1 partial
concat, and stitching output blocks. All FLOPs (reduction, quantization,
matmul, rescale) run on device.

The matmul phase runs at the power-capped PE roofline: the board GPIO
throttler holds the PE at K=13/16 of 2.4GHz under sustained 8-core load,
so 34.4 GFLOP/core needs ~538us of PE time (measured 534us, >97%
occupancy in the steady loop).
"""

import os
import sys
import types

import numpy as np


def _ensure_axon_hooks_module():
    """Some images lack ``antenv.axon_hooks``; ``run_bass_kernel_spmd`` imports
    it unconditionally when tracing is requested. Install a no-op fallback so a
    BASS_TRACE=1 environment degrades to "no trace" instead of crashing."""
    try:
        import antenv.axon_hooks  # noqa: F401
        return
    except ImportError:
        pass
    try:
        import antenv
    except ImportError:
        return
    mod = types.ModuleType("antenv.axon_hooks")
    mod._hook = None

    def set_axon_ntff_profile_hook(h):
        mod._hook = h

    def get_axon_ntff_profile_hook():
        return mod._hook

    mod.set_axon_ntff_profile_hook = set_axon_ntff_profile_hook
    mod.get_axon_ntff_profile_hook = get_axon_ntff_profile_hook
    sys.modules["antenv.axon_hooks"] = mod
    antenv.axon_hooks = mod


_ensure_axon_hooks_module()

# ---- problem constants (hardcoded per contract) ----
B, SEQ, I_DIM, O_DIM = 4, 2048, 4096, 4096
S_TOT = B * SEQ            # 8192
R_CORES, C_CORES = 2, 4    # grid: batch x out_features
N_CORES = R_CORES * C_CORES
S_CORE = S_TOT // R_CORES  # 4096 sequence rows per core
O_CORE = O_DIM // C_CORES  # 1024 output features per core
P = 128
KP = I_DIM // P            # 32 contraction chunks
S_BLK = 256                # s columns per x load block
N_SBLK = S_CORE // S_BLK   # 16
W_RED = O_DIM // N_CORES   # 512: rows of W reduced per core for mean|W|
RT = W_RED // P            # 4 reduction chunks in pass 1
WCH = 2                    # ko chunks per W staging tile ([128, 2, 1024] = 1 MB)
N_WT = KP // WCH           # 16 stage/quantize tiles
MAGIC = 1.5 * (2.0 ** 23)  # fp32 round-to-nearest-even trick constant
EPS = 1e-5
GATHER = os.environ.get("BITLIN_GATHER", "twopass")

_nc_cache = {}


def _build_reduce_kernel():
    """Pass 1: spart[p] = sum over its 1/8 slice of |W| (per-partition)."""
    import concourse.mybir as mybir
    import concourse.tile as tile
    from concourse import bacc

    f32 = mybir.dt.float32
    Alu = mybir.AluOpType
    Act = mybir.ActivationFunctionType

    nc = bacc.Bacc(
        "TRN2",
        target_bir_lowering=False,
        debug=False,
        enable_asserts=False,
        num_devices=N_CORES,
    )
    # row-major [512, 4096] slice: 16 KB contiguous per (partition, chunk)
    wredr = nc.dram_tensor("wredr", [W_RED, I_DIM], f32, kind="ExternalInput")
    spart = nc.dram_tensor("spart", [P, 1], f32, kind="ExternalOutput")
    wr = wredr.ap().rearrange("(t p) i -> p t i", p=P)  # [128, 4, 4096]

    with tile.TileContext(nc) as tc:
        with (
            tc.tile_pool(name="wst", bufs=4) as wst,
            tc.tile_pool(name="st", bufs=1) as st,
        ):
            part = st.tile([P, 2], f32)
            for t in range(2):
                wt = wst.tile([P, 2, I_DIM], f32, tag="w")
                nc.sync.dma_start(wt[:], wr[:, 2 * t : 2 * t + 2, :])
                if t == 0:
                    nc.vector.tensor_reduce(
                        part[:, t : t + 1],
                        wt[:],
                        axis=mybir.AxisListType.XY,
                        op=Alu.add,
                        apply_absolute_value=True,
                    )
                else:
                    nc.scalar.activation(
                        wt[:], wt[:], Act.Abs, accum_out=part[:, t : t + 1]
                    )
            accv = st.tile([P, 1], f32)
            nc.vector.tensor_reduce(
                accv[:], part[:], axis=mybir.AxisListType.X, op=Alu.add
            )
            nc.sync.dma_start(spart.ap()[:, :], accv[:])

    nc.compile()
    return nc


def _build_kernel():
    import concourse.mybir as mybir
    import concourse.tile as tile
    from concourse import bacc
    from concourse.tile import add_dep_helper

    f32 = mybir.dt.float32
    bf16 = mybir.dt.bfloat16
    Alu = mybir.AluOpType
    Act = mybir.ActivationFunctionType

    nc = bacc.Bacc(
        "TRN2",
        target_bir_lowering=False,
        debug=False,
        enable_asserts=False,
        num_devices=N_CORES,
    )

    twopass = GATHER == "twopass"
    xT = nc.dram_tensor("xT", [I_DIM, S_CORE], f32, kind="ExternalInput")
    wT = nc.dram_tensor("wT", [I_DIM, O_CORE], f32, kind="ExternalInput")
    if twopass:
        spart = nc.dram_tensor("spart", [P, N_CORES], f32, kind="ExternalInput")
    else:
        wred = nc.dram_tensor("wred", [I_DIM, W_RED], f32, kind="ExternalInput")
        wred_r = wred.ap().rearrange("(ko p) o -> p ko o", p=P)  # [128, 32, 512]
    y = nc.dram_tensor("y", [S_CORE, O_CORE], f32, kind="ExternalOutput")

    xT_r = xT.ap().rearrange("(ko p) s -> p ko s", p=P)    # [128, 32, 4096]
    wT_r = wT.ap().rearrange("(ko p) o -> p ko o", p=P)    # [128, 32, 1024]
    y_ap = y.ap()

    with tile.TileContext(nc) as tc:
        with (
            tc.tile_pool(name="const", bufs=1) as const_pool,
            tc.tile_pool(name="stats", bufs=1) as stats,
            tc.tile_pool(name="wstage", bufs=7) as wstage,
            tc.tile_pool(name="wq", bufs=1) as wq_pool,
            tc.tile_pool(name="xbf", bufs=3) as xbf_pool,
            tc.tile_pool(name="yout", bufs=3) as yout_pool,
            tc.tile_pool(name="psum_s", bufs=1, space="PSUM") as psum_s,
            tc.tile_pool(name="psum_mm", bufs=3, space="PSUM") as psum_mm,
            tc.tile_pool(name="dram", bufs=1, space="DRAM") as dram_pool,
        ):
            wq_tiles = [
                wq_pool.tile([P, WCH, O_CORE], bf16, tag=f"wq{t}", name=f"wq{t}")
                for t in range(N_WT)
            ]

            # ---------- Phase A/B: per-core |W| partials -> global sum ----------
            bounce_dma = None
            gate = None
            if twopass:
                # partials were computed in pass 1; every core got the same
                # [128, 8] block. ~3us DMA + ~1us reduce.
                spart_sb = stats.tile([P, N_CORES], f32)
                nc.sync.dma_start(spart_sb[:], spart.ap())
                acc_r = stats.tile([P, 1], f32)
                nc.vector.tensor_reduce(
                    acc_r[:], spart_sb[:], axis=mybir.AxisListType.X, op=Alu.add
                )
            else:
                # single-kernel path: reduce the wred slice here, AllGather.
                n_rtiles = KP // 4  # 8 tiles [128, 4, 512] = 1 MB each
                red_all = stats.tile([P, n_rtiles], f32)
                for t in range(n_rtiles):
                    wt = wstage.tile([P, 4, W_RED], f32, tag="wstage")
                    nc.sync.dma_start(wt[:], wred_r[:, t * 4 : (t + 1) * 4, :])
                    if t % 2 == 0:
                        nc.vector.tensor_reduce(
                            red_all[:, t : t + 1],
                            wt[:],
                            axis=mybir.AxisListType.XY,
                            op=Alu.add,
                            apply_absolute_value=True,
                        )
                    else:
                        nc.scalar.activation(
                            wt[:], wt[:], Act.Abs, accum_out=red_all[:, t : t + 1]
                        )
                acc = stats.tile([P, 1], f32)
                nc.vector.tensor_reduce(
                    acc[:], red_all[:], axis=mybir.AxisListType.X, op=Alu.add
                )
                cc_in = dram_pool.tile([P, 1], f32)
                cc_out = dram_pool.tile([N_CORES * P, 1], f32, addr_space="Shared")
                bounce_dma = nc.sync.dma_start(cc_in[:], acc[:])
                gate = nc.gpsimd.collective_compute(
                    "AllGather",
                    Alu.bypass,
                    replica_groups=[list(range(N_CORES))],
                    ins=[cc_in.opt()],
                    outs=[cc_out.opt()],
                )
                # read back as [128, 8]: partition p, free r <- dram[r*128 + p].
                # Keep this per-partition tree reduction: a flat 1024-element
                # sequential sum lands measurably further from the reference's
                # fp32 summation.
                acc_g = stats.tile([P, N_CORES], f32)
                nc.sync.dma_start(
                    acc_g[:], cc_out.rearrange("(r p) one -> p (r one)", p=P)
                )
                acc_r = stats.tile([P, 1], f32)
                nc.vector.tensor_reduce(
                    acc_r[:], acc_g[:], axis=mybir.AxisListType.X, op=Alu.add
                )

            # ---------- Phase C: scale scalars, broadcast to all partitions ----------
            # global sum broadcast: ones^T @ acc_r -> every partition = full
            # sum (the framework emits the exact 2-pass fp32 PE mode here)
            inv_numel = 1.0 / (float(I_DIM) * float(O_DIM))
            ones_b = const_pool.tile([P, P], f32)
            nc.vector.memset(ones_b[:], 1.0)
            ps_b = psum_s.tile([P, 1], f32)
            nc.tensor.matmul(ps_b[:], lhsT=ones_b[:], rhs=acc_r[:], start=True, stop=True)

            # sinv first: it gates quantization (scale_t is only needed at
            # output eviction, much later)
            seps_t = stats.tile([P, 1], f32)   # scale + eps
            nc.vector.tensor_scalar(
                seps_t[:], ps_b[:], inv_numel, EPS, op0=Alu.mult, op1=Alu.add
            )
            sinv_t = stats.tile([P, 1], f32)   # 1 / (scale + eps)
            nc.vector.reciprocal(sinv_t[:], seps_t[:])
            scale_t = stats.tile([P, 1], f32)  # mean(|W|)
            nc.vector.tensor_scalar_mul(scale_t[:], ps_b[:], inv_numel)

            # ---------- Phase D: quantize W -> bf16 integers (DVE + ACT split) ----------
            # single pass over wT, staged in plain order on the Sync HWDGE
            # queue only (issuing DMAs from nc.scalar serializes the triggers
            # behind the ACT compute chain on the same engine FIFO). Parity
            # split across DVE/ACT so both engines chew arrivals concurrently.
            first_done = False
            wt_dmas = []
            for t in range(N_WT):
                wt = wstage.tile([P, WCH, O_CORE], f32, tag="wstage")
                dma = nc.sync.dma_start(wt[:], wT_r[:, t * WCH : (t + 1) * WCH, :])
                wt_dmas.append(dma)
                if not first_done and bounce_dma is not None:
                    first_done = True
                    # cc path: keep wred DMAs exclusive on the queue until the
                    # collective input is on its way
                    add_dep_helper(dma.ins, bounce_dma.ins, sync=False,
                                   reason="stage wT after AR input bounce")
                if t % 2 == 0:
                    # wn = W * (1/(scale+eps)) + MAGIC  (fp32, in place)
                    nc.vector.tensor_scalar(
                        wt[:], wt[:], sinv_t[:], MAGIC, op0=Alu.mult, op1=Alu.add
                    )
                    # wq = (wn - MAGIC) cast to bf16  (exact small integers)
                    nc.vector.tensor_scalar_sub(wq_tiles[t][:], wt[:], MAGIC)
                else:
                    nc.scalar.activation(
                        wt[:], wt[:], Act.Copy, bias=MAGIC, scale=sinv_t[:]
                    )
                    nc.scalar.activation(
                        wq_tiles[t][:], wt[:], Act.Copy, bias=-MAGIC, scale=1.0
                    )

            # ---------- Phase E: y = (x @ Wq^T) * scale ----------
            def evict(ps0, ps1, row):
                yo = yout_pool.tile([P, O_CORE], f32, name="yo")
                nc.vector.tensor_scalar_mul(yo[:, 0:512], ps0[:], scale_t[:])
                nc.vector.tensor_scalar_mul(yo[:, 512:1024], ps1[:], scale_t[:])
                nc.sync.dma_start(y_ap[row : row + P, :], yo[:])

            x_blocks = []
            for nb in range(3):
                xb = xbf_pool.tile([P, KP, S_BLK], bf16, tag="xb", name=f"xb{nb}")
                # SWDGE casts fp32 -> bf16 inline during the HBM->SBUF DMA
                xdma = nc.gpsimd.dma_start(
                    xb[:], xT_r[:, :, nb * S_BLK : (nb + 1) * S_BLK]
                )
                if gate is not None:
                    # don't let x descriptor-gen delay the gather trigger on
                    # the gpsimd queue
                    add_dep_helper(xdma.ins, gate.ins, sync=False,
                                   reason="x load after gather trigger")
                elif nb > 0:
                    # x block 0 overlaps the wT stream (the fast path gates on
                    # it ~10us after the scale); blocks 1-2 wait for most of
                    # the quantization-gating wT stream to LAND (real sem dep:
                    # an issue-order dep would still let their SDMA packets
                    # round-robin against wT and halve its bandwidth — that
                    # pushed the first fast-path matmul from ~20us to ~67us)
                    add_dep_helper(xdma.ins, wt_dmas[4 + 4 * nb].ins, sync=True,
                                   reason="x prefetch behind wT stream")
                x_blocks.append(xb)

            # Fast path: s-tiles 0 and 1 accumulate in 4 concurrent PSUM banks,
            # consuming wq tiles in staging order so the PE starts on the first
            # quantized tile. Only x block 0 is required: gating any fast-path
            # matmul on later x blocks stalls the in-order PE queue at t=0.
            fast_units = []  # (psum, s_tile, o_half)
            for stg in range(2):
                ps0 = psum_mm.tile([P, 512], f32, tag="mm0", name=f"fps0_{stg}")
                ps1 = psum_mm.tile([P, 512], f32, tag="mm1", name=f"fps1_{stg}")
                fast_units.append((ps0, stg, 0))
                fast_units.append((ps1, stg, 1))
            for t in range(N_WT):
                first, last = (t == 0), (t == N_WT - 1)
                for ps, stg, half in fast_units:
                    s_lo = stg * P
                    for kk in range(WCH):
                        k = t * WCH + kk
                        nc.tensor.matmul(
                            ps[:],
                            lhsT=x_blocks[0][:, k, s_lo : s_lo + P],
                            rhs=wq_tiles[t][:, kk, 512 * half : 512 * (half + 1)],
                            start=first and kk == 0,
                            stop=last and kk == WCH - 1,
                        )
            for stg in range(2):
                evict(fast_units[2 * stg][0], fast_units[2 * stg + 1][0], stg * P)

            # Steady state (s-tiles 2..31)
            for nb in range(1, N_SBLK):
                if nb >= 3:
                    xb = xbf_pool.tile([P, KP, S_BLK], bf16, tag="xb", name=f"xb{nb}")
                    nc.gpsimd.dma_start(
                        xb[:], xT_r[:, :, nb * S_BLK : (nb + 1) * S_BLK]
                    )
                else:
                    xb = x_blocks[nb]
                for st in range(S_BLK // P):
                    ps0 = psum_mm.tile([P, 512], f32, tag="mm0", name="ps0")
                    ps1 = psum_mm.tile([P, 512], f32, tag="mm1", name="ps1")
                    s_lo = st * P
                    for k in range(KP):
                        lhs = xb[:, k, s_lo : s_lo + P]
                        wqk = wq_tiles[k // WCH][:, k % WCH, :]
                        first, last = (k == 0), (k == KP - 1)
                        nc.tensor.matmul(
                            ps0[:], lhsT=lhs, rhs=wqk[:, 0:512],
                            start=first, stop=last,
                        )
                        nc.tensor.matmul(
                            ps1[:], lhsT=lhs, rhs=wqk[:, 512:1024],
                            start=first, stop=last,
                        )
                    evict(ps0, ps1, nb * S_BLK + s_lo)

    nc.compile()
    return nc


def _get_nc():
    if "nc" not in _nc_cache:
        _nc_cache["nc"] = _build_kernel()
    return _nc_cache["nc"]


def _get_nc_reduce():
    if "nc_red" not in _nc_cache:
        _nc_cache["nc_red"] = _build_reduce_kernel()
    return _nc_cache["nc_red"]


def _shard_inputs(x, W, spart=None):
    x2 = np.ascontiguousarray(np.asarray(x, dtype=np.float32).reshape(S_TOT, I_DIM))
    W2 = np.ascontiguousarray(np.asarray(W, dtype=np.float32))

    xT_slices = [
        np.ascontiguousarray(x2[r * S_CORE : (r + 1) * S_CORE, :].T)
        for r in range(R_CORES)
    ]
    wT_slices = [
        np.ascontiguousarray(W2[c * O_CORE : (c + 1) * O_CORE, :].T)
        for c in range(C_CORES)
    ]
    if GATHER == "cc":
        wred_slices = [
            np.ascontiguousarray(W2[c * W_RED : (c + 1) * W_RED, :].T)
            for c in range(N_CORES)
        ]
    in_maps = []
    for core in range(N_CORES):
        ri, ci = core // C_CORES, core % C_CORES
        m = {"xT": xT_slices[ri], "wT": wT_slices[ci]}
        if GATHER == "cc":
            m["wred"] = wred_slices[core]
        elif spart is not None:
            m["spart"] = spart
        in_maps.append(m)
    return in_maps


def _gather_output(results):
    y = np.empty((S_TOT, O_DIM), dtype=np.float32)
    for core in range(N_CORES):
        ri, ci = core // C_CORES, core % C_CORES
        y[ri * S_CORE : (ri + 1) * S_CORE, ci * O_CORE : (ci + 1) * O_CORE] = (
            results[core]["y"]
        )
    return y.reshape(B, SEQ, O_DIM)


def _prime_axon_profile():
    """Refresh the axon profile side-channel: one tiny device execute plus a
    start/stop pair. `axon_start_nrt_profile` returns -1 unless the client has
    been active recently, so this runs right before the profiled execute."""
    try:
        import ctypes
        import tempfile

        import jax
        import jax.numpy as jnp

        np.asarray(jax.jit(lambda a: a + 1)(jnp.zeros((8,))))
        lib = ctypes.CDLL("/opt/axon/libaxon_pjrt.so")
        lib.axon_start_nrt_profile.argtypes = [
            ctypes.POINTER(ctypes.c_int64),
            ctypes.c_size_t,
        ]
        lib.axon_start_nrt_profile.restype = ctypes.c_int64
        lib.axon_stop_nrt_profile.argtypes = [ctypes.c_char_p]
        lib.axon_stop_nrt_profile.restype = ctypes.c_int64
        ids = (ctypes.c_int64 * 1)(0)
        rc = lib.axon_start_nrt_profile(ids, 1)
        if rc == 0:
            lib.axon_stop_nrt_profile(tempfile.mkdtemp().encode())
        print(f"axon profile primed (rc={rc})")
    except Exception as e:
        print(f"axon profile priming failed: {type(e).__name__}: {e}")


def _run_reduce(W, **spmd_kwargs):
    """Pass 1: per-core |W|-slice partials. Host only concatenates."""
    from concourse.bass_utils import run_bass_kernel_spmd

    nc1 = _get_nc_reduce()
    W2 = np.ascontiguousarray(np.asarray(W, dtype=np.float32))
    in_maps = [
        {"wredr": np.ascontiguousarray(W2[c * W_RED : (c + 1) * W_RED, :])}
        for c in range(N_CORES)
    ]
    res = run_bass_kernel_spmd(
        nc1, in_maps, core_ids=list(range(N_CORES)), **spmd_kwargs
    )
    spart = np.ascontiguousarray(
        np.concatenate([res.results[c]["spart"] for c in range(N_CORES)], axis=1)
    )  # [128, 8]
    return spart, res


def _run(x, W, **spmd_kwargs):
    import time

    from concourse.bass_utils import run_bass_kernel_spmd

    nc = _get_nc()
    last_err = None
    for attempt in range(3):
        _prime_axon_profile()
        try:
            res1 = None
            spart = None
            if GATHER == "twopass":
                spart, res1 = _run_reduce(W, **spmd_kwargs)
            in_maps = _shard_inputs(x, W, spart=spart)
            res = run_bass_kernel_spmd(
                nc, in_maps, core_ids=list(range(N_CORES)), **spmd_kwargs
            )
            res.reduce_pass = res1
            return _gather_output(res.results), res
        except Exception as e:  # transient device wedges recover on retry
            last_err = e
            time.sleep(5.0 * (attempt + 1))
    raise last_err


def kernel(x, W):
    out, _ = _run(x, W)
    return out


# revision 11
# speedup vs baseline: 1.2434x; 1.0394x over previous
"""BitLinear (ternary-quantized linear) kernel for Trainium2, 8 NeuronCores.

Reference computation:
    scale = mean(|W|);  Wq = round(W / (scale + 1e-5));  y = (x @ Wq^T) * scale

Distribution (2x4 grid over 8 cores):
  - batch/sequence dim (8192 rows of x) split 2 ways  -> ri = core // 4
  - out_features dim (4096 rows of W) split 4 ways    -> ci = core % 4
  Each core computes y block [4096 s, 1024 o].

The global mean(|W|) must be EXACT: quantizing with a per-core quarter
mean measures rel_err 2.67e-2 on the harness inputs (~3.6k of 16.7M
weights flip their rounding boundary) vs the 2e-2 gate. Exact-scale
bf16 measures 1.72e-3.

Scale strategies (BITLIN_GATHER):
  "twopass" (default): two NEFF executions.
      Pass 1 (~40us): each core reduces a distinct 1/8 of W (row-major
      slice -> 16KB/partition DMA rows at line rate) to a [128,1]
      fp32 partial. The host only CONCATENATES the 8 partials (layout,
      no FLOPs) and feeds the [128,8] block to every core.
      Pass 2 (~570us): reduce the replicated partials on-device (~2us),
      broadcast via ones-matmul (full-precision fp32 2-pass PE mode),
      then stream wT once: quantize on arrival (DVE/ACT parity split),
      matmuls from ~20us in wq arrival order.
  "cc": single kernel with an ncfw AllGather combining the partials.
      The ncfw path has a hard ~110us scale floor on this stack (TOPSP
      wakeup barrier ~50us starting at ~21us + trigger + ~26us Mesh
      AllGather for 512B): 671us total measured.
  Abandoned: peer-SBUF SDMA gather deadlocks on HW (core launch skew
  exceeds the send offset; fixed-threshold semaphore handshakes lose
  increments and wedge the exec unit). Dummy warm-up AllGather: ncfw
  serializes collectives on one stream, pushing the real one later
  (697us measured).

Host side does layout only: x^T / W^T slices (the contraction dim i must
be the SBUF partition dim on both matmul operands), the pass-1 partial
concat, and stitching output blocks. All FLOPs (reduction, quantization,
matmul, rescale) run on device.

The matmul phase runs at the power-capped PE roofline: the board GPIO
throttler holds the PE at K=13/16 of 2.4GHz under sustained 8-core load,
so 34.4 GFLOP/core needs ~538us of PE time (measured 534us, >97%
occupancy in the steady loop).
"""

import os
import sys
import types

import numpy as np


def _ensure_axon_hooks_module():
    """Some images lack ``antenv.axon_hooks``; ``run_bass_kernel_spmd`` imports
    it unconditionally when tracing is requested. Install a no-op fallback so a
    BASS_TRACE=1 environment degrades to "no trace" instead of crashing."""
    try:
        import antenv.axon_hooks  # noqa: F401
        return
    except ImportError:
        pass
    try:
        import antenv
    except ImportError:
        return
    mod = types.ModuleType("antenv.axon_hooks")
    mod._hook = None

    def set_axon_ntff_profile_hook(h):
        mod._hook = h

    def get_axon_ntff_profile_hook():
        return mod._hook

    mod.set_axon_ntff_profile_hook = set_axon_ntff_profile_hook
    mod.get_axon_ntff_profile_hook = get_axon_ntff_profile_hook
    sys.modules["antenv.axon_hooks"] = mod
    antenv.axon_hooks = mod


_ensure_axon_hooks_module()

# ---- problem constants (hardcoded per contract) ----
B, SEQ, I_DIM, O_DIM = 4, 2048, 4096, 4096
S_TOT = B * SEQ            # 8192
R_CORES, C_CORES = 2, 4    # grid: batch x out_features
N_CORES = R_CORES * C_CORES
S_CORE = S_TOT // R_CORES  # 4096 sequence rows per core
O_CORE = O_DIM // C_CORES  # 1024 output features per core
P = 128
KP = I_DIM // P            # 32 contraction chunks
S_BLK = 256                # s columns per x load block
N_SBLK = S_CORE // S_BLK   # 16
W_RED = O_DIM // N_CORES   # 512: rows of W reduced per core for mean|W|
RT = W_RED // P            # 4 reduction chunks in pass 1
WCH = 2                    # ko chunks per W staging tile ([128, 2, 1024] = 1 MB)
N_WT = KP // WCH           # 16 stage/quantize tiles
MAGIC = 1.5 * (2.0 ** 23)  # fp32 round-to-nearest-even trick constant
EPS = 1e-5
GATHER = os.environ.get("BITLIN_GATHER", "twopass")

_nc_cache = {}


def _build_reduce_kernel():
    """Pass 1: spart[p] = sum over its 1/8 slice of |W| (per-partition)."""
    import concourse.mybir as mybir
    import concourse.tile as tile
    from concourse import bacc

    f32 = mybir.dt.float32
    Alu = mybir.AluOpType
    Act = mybir.ActivationFunctionType

    nc = bacc.Bacc(
        "TRN2",
        target_bir_lowering=False,
        debug=False,
        enable_asserts=False,
        num_devices=N_CORES,
    )
    # row-major [512, 4096] slice: 16 KB contiguous per (partition, chunk)
    wredr = nc.dram_tensor("wredr", [W_RED, I_DIM], f32, kind="ExternalInput")
    spart = nc.dram_tensor("spart", [P, 1], f32, kind="ExternalOutput")
    wr = wredr.ap().rearrange("(t p) i -> p t i", p=P)  # [128, 4, 4096]

    with tile.TileContext(nc) as tc:
        with (
            tc.tile_pool(name="wst", bufs=4) as wst,
            tc.tile_pool(name="st", bufs=1) as st,
        ):
            part = st.tile([P, 2], f32)
            for t in range(2):
                wt = wst.tile([P, 2, I_DIM], f32, tag="w")
                nc.sync.dma_start(wt[:], wr[:, 2 * t : 2 * t + 2, :])
                if t == 0:
                    nc.vector.tensor_reduce(
                        part[:, t : t + 1],
                        wt[:],
                        axis=mybir.AxisListType.XY,
                        op=Alu.add,
                        apply_absolute_value=True,
                    )
                else:
                    nc.scalar.activation(
                        wt[:], wt[:], Act.Abs, accum_out=part[:, t : t + 1]
                    )
            accv = st.tile([P, 1], f32)
            nc.vector.tensor_reduce(
                accv[:], part[:], axis=mybir.AxisListType.X, op=Alu.add
            )
            nc.sync.dma_start(spart.ap()[:, :], accv[:])

    nc.compile()
    return nc


def _build_kernel():
    import concourse.mybir as mybir
    import concourse.tile as tile
    from concourse import bacc
    from concourse.tile import add_dep_helper

    f32 = mybir.dt.float32
    bf16 = mybir.dt.bfloat16
    Alu = mybir.AluOpType
    Act = mybir.ActivationFunctionType

    nc = bacc.Bacc(
        "TRN2",
        target_bir_lowering=False,
        debug=False,
        enable_asserts=False,
        num_devices=N_CORES,
    )

    twopass = GATHER == "twopass"
    # x arrives block-contiguous: [nb, p, ko, s'] so one x block is 128
    # partition-contiguous 32 KB runs. The naive [I, S] column-slice layout
    # fragments each 4 MiB block into 4096x1KB descriptors — measured ~60us
    # per block under wT contention, which stalled the first matmul to ~72us.
    x5 = nc.dram_tensor("x5", [N_SBLK, P, KP, S_BLK], f32, kind="ExternalInput")
    wT = nc.dram_tensor("wT", [I_DIM, O_CORE], f32, kind="ExternalInput")
    if twopass:
        spart = nc.dram_tensor("spart", [P, N_CORES], f32, kind="ExternalInput")
    else:
        wred = nc.dram_tensor("wred", [I_DIM, W_RED], f32, kind="ExternalInput")
        wred_r = wred.ap().rearrange("(ko p) o -> p ko o", p=P)  # [128, 32, 512]
    y = nc.dram_tensor("y", [S_CORE, O_CORE], f32, kind="ExternalOutput")

    x5_ap = x5.ap()                                        # [16, 128, 32, 256]
    wT_r = wT.ap().rearrange("(ko p) o -> p ko o", p=P)    # [128, 32, 1024]
    y_ap = y.ap()

    with tile.TileContext(nc) as tc:
        with (
            tc.tile_pool(name="const", bufs=1) as const_pool,
            tc.tile_pool(name="stats", bufs=1) as stats,
            tc.tile_pool(name="wstage", bufs=7) as wstage,
            tc.tile_pool(name="wq", bufs=1) as wq_pool,
            tc.tile_pool(name="xbf", bufs=3) as xbf_pool,
            tc.tile_pool(name="yout", bufs=3) as yout_pool,
            tc.tile_pool(name="psum_s", bufs=1, space="PSUM") as psum_s,
            tc.tile_pool(name="psum_mm", bufs=3, space="PSUM") as psum_mm,
            tc.tile_pool(name="dram", bufs=1, space="DRAM") as dram_pool,
        ):
            wq_tiles = [
                wq_pool.tile([P, WCH, O_CORE], bf16, tag=f"wq{t}", name=f"wq{t}")
                for t in range(N_WT)
            ]

            # ---------- Phase A/B: per-core |W| partials -> global sum ----------
            bounce_dma = None
            gate = None
            if twopass:
                # partials were computed in pass 1; every core got the same
                # [128, 8] block. ~3us DMA + ~1us reduce.
                spart_sb = stats.tile([P, N_CORES], f32)
                nc.sync.dma_start(spart_sb[:], spart.ap())
                acc_r = stats.tile([P, 1], f32)
                nc.vector.tensor_reduce(
                    acc_r[:], spart_sb[:], axis=mybir.AxisListType.X, op=Alu.add
                )
            else:
                # single-kernel path: reduce the wred slice here, AllGather.
                n_rtiles = KP // 4  # 8 tiles [128, 4, 512] = 1 MB each
                red_all = stats.tile([P, n_rtiles], f32)
                for t in range(n_rtiles):
                    wt = wstage.tile([P, 4, W_RED], f32, tag="wstage")
                    nc.sync.dma_start(wt[:], wred_r[:, t * 4 : (t + 1) * 4, :])
                    if t % 2 == 0:
                        nc.vector.tensor_reduce(
                            red_all[:, t : t + 1],
                            wt[:],
                            axis=mybir.AxisListType.XY,
                            op=Alu.add,
                            apply_absolute_value=True,
                        )
                    else:
                        nc.scalar.activation(
                            wt[:], wt[:], Act.Abs, accum_out=red_all[:, t : t + 1]
                        )
                acc = stats.tile([P, 1], f32)
                nc.vector.tensor_reduce(
                    acc[:], red_all[:], axis=mybir.AxisListType.X, op=Alu.add
                )
                cc_in = dram_pool.tile([P, 1], f32)
                cc_out = dram_pool.tile([N_CORES * P, 1], f32, addr_space="Shared")
                bounce_dma = nc.sync.dma_start(cc_in[:], acc[:])
                gate = nc.gpsimd.collective_compute(
                    "AllGather",
                    Alu.bypass,
                    replica_groups=[list(range(N_CORES))],
                    ins=[cc_in.opt()],
                    outs=[cc_out.opt()],
                )
                # read back as [128, 8]: partition p, free r <- dram[r*128 + p].
                # Keep this per-partition tree reduction: a flat 1024-element
                # sequential sum lands measurably further from the reference's
                # fp32 summation.
                acc_g = stats.tile([P, N_CORES], f32)
                nc.sync.dma_start(
                    acc_g[:], cc_out.rearrange("(r p) one -> p (r one)", p=P)
                )
                acc_r = stats.tile([P, 1], f32)
                nc.vector.tensor_reduce(
                    acc_r[:], acc_g[:], axis=mybir.AxisListType.X, op=Alu.add
                )

            # ---------- Phase C: scale scalars, broadcast to all partitions ----------
            # global sum broadcast: ones^T @ acc_r -> every partition = full
            # sum (the framework emits the exact 2-pass fp32 PE mode here)
            inv_numel = 1.0 / (float(I_DIM) * float(O_DIM))
            ones_b = const_pool.tile([P, P], f32)
            nc.vector.memset(ones_b[:], 1.0)
            ps_b = psum_s.tile([P, 1], f32)
            nc.tensor.matmul(ps_b[:], lhsT=ones_b[:], rhs=acc_r[:], start=True, stop=True)

            # sinv first: it gates quantization (scale_t is only needed at
            # output eviction, much later)
            seps_t = stats.tile([P, 1], f32)   # scale + eps
            nc.vector.tensor_scalar(
                seps_t[:], ps_b[:], inv_numel, EPS, op0=Alu.mult, op1=Alu.add
            )
            sinv_t = stats.tile([P, 1], f32)   # 1 / (scale + eps)
            nc.vector.reciprocal(sinv_t[:], seps_t[:])
            scale_t = stats.tile([P, 1], f32)  # mean(|W|)
            nc.vector.tensor_scalar_mul(scale_t[:], ps_b[:], inv_numel)

            # ---------- Phase D: quantize W -> bf16 integers (DVE + ACT split) ----------
            # single pass over wT, staged in plain order on the Sync HWDGE
            # queue only (issuing DMAs from nc.scalar serializes the triggers
            # behind the ACT compute chain on the same engine FIFO). Parity
            # split across DVE/ACT so both engines chew arrivals concurrently.
            first_done = False
            wt_dmas = []
            for t in range(N_WT):
                wt = wstage.tile([P, WCH, O_CORE], f32, tag="wstage")
                dma = nc.sync.dma_start(wt[:], wT_r[:, t * WCH : (t + 1) * WCH, :])
                wt_dmas.append(dma)
                if not first_done and bounce_dma is not None:
                    first_done = True
                    # cc path: keep wred DMAs exclusive on the queue until the
                    # collective input is on its way
                    add_dep_helper(dma.ins, bounce_dma.ins, sync=False,
                                   reason="stage wT after AR input bounce")
                if t % 2 == 0:
                    # wn = W * (1/(scale+eps)) + MAGIC  (fp32, in place)
                    nc.vector.tensor_scalar(
                        wt[:], wt[:], sinv_t[:], MAGIC, op0=Alu.mult, op1=Alu.add
                    )
                    # wq = (wn - MAGIC) cast to bf16  (exact small integers)
                    nc.vector.tensor_scalar_sub(wq_tiles[t][:], wt[:], MAGIC)
                else:
                    nc.scalar.activation(
                        wt[:], wt[:], Act.Copy, bias=MAGIC, scale=sinv_t[:]
                    )
                    nc.scalar.activation(
                        wq_tiles[t][:], wt[:], Act.Copy, bias=-MAGIC, scale=1.0
                    )

            # ---------- Phase E: y = (x @ Wq^T) * scale ----------
            def evict(ps0, ps1, row):
                yo = yout_pool.tile([P, O_CORE], f32, name="yo")
                nc.vector.tensor_scalar_mul(yo[:, 0:512], ps0[:], scale_t[:])
                nc.vector.tensor_scalar_mul(yo[:, 512:1024], ps1[:], scale_t[:])
                nc.sync.dma_start(y_ap[row : row + P, :], yo[:])

            x_blocks = []
            for nb in range(3):
                xb = xbf_pool.tile([P, KP, S_BLK], bf16, tag="xb", name=f"xb{nb}")
                # SWDGE casts fp32 -> bf16 inline during the HBM->SBUF DMA
                xdma = nc.gpsimd.dma_start(xb[:], x5_ap[nb])
                if gate is not None:
                    # don't let x descriptor-gen delay the gather trigger on
                    # the gpsimd queue
                    add_dep_helper(xdma.ins, gate.ins, sync=False,
                                   reason="x load after gather trigger")
                elif nb > 0:
                    # x block 0 overlaps the wT stream (the fast path gates on
                    # it ~10us after the scale); blocks 1-2 wait for most of
                    # the quantization-gating wT stream to LAND (real sem dep:
                    # an issue-order dep would still let their SDMA packets
                    # round-robin against wT and halve its bandwidth — that
                    # pushed the first fast-path matmul from ~20us to ~67us)
                    add_dep_helper(xdma.ins, wt_dmas[4 + 4 * nb].ins, sync=True,
                                   reason="x prefetch behind wT stream")
                x_blocks.append(xb)

            # Fast path: s-tiles 0 and 1 accumulate in 4 concurrent PSUM banks,
            # consuming wq tiles in staging order so the PE starts on the first
            # quantized tile. Only x block 0 is required: gating any fast-path
            # matmul on later x blocks stalls the in-order PE queue at t=0.
            fast_units = []  # (psum, s_tile, o_half)
            for stg in range(2):
                ps0 = psum_mm.tile([P, 512], f32, tag="mm0", name=f"fps0_{stg}")
                ps1 = psum_mm.tile([P, 512], f32, tag="mm1", name=f"fps1_{stg}")
                fast_units.append((ps0, stg, 0))
                fast_units.append((ps1, stg, 1))
            for t in range(N_WT):
                first, last = (t == 0), (t == N_WT - 1)
                for ps, stg, half in fast_units:
                    s_lo = stg * P
                    for kk in range(WCH):
                        k = t * WCH + kk
                        nc.tensor.matmul(
                            ps[:],
                            lhsT=x_blocks[0][:, k, s_lo : s_lo + P],
                            rhs=wq_tiles[t][:, kk, 512 * half : 512 * (half + 1)],
                            start=first and kk == 0,
                            stop=last and kk == WCH - 1,
                        )
            for stg in range(2):
                evict(fast_units[2 * stg][0], fast_units[2 * stg + 1][0], stg * P)

            # Steady state (s-tiles 2..31)
            for nb in range(1, N_SBLK):
                if nb >= 3:
                    xb = xbf_pool.tile([P, KP, S_BLK], bf16, tag="xb", name=f"xb{nb}")
                    nc.gpsimd.dma_start(xb[:], x5_ap[nb])
                else:
                    xb = x_blocks[nb]
                for st in range(S_BLK // P):
                    ps0 = psum_mm.tile([P, 512], f32, tag="mm0", name="ps0")
                    ps1 = psum_mm.tile([P, 512], f32, tag="mm1", name="ps1")
                    s_lo = st * P
                    for k in range(KP):
                        lhs = xb[:, k, s_lo : s_lo + P]
                        wqk = wq_tiles[k // WCH][:, k % WCH, :]
                        first, last = (k == 0), (k == KP - 1)
                        nc.tensor.matmul(
                            ps0[:], lhsT=lhs, rhs=wqk[:, 0:512],
                            start=first, stop=last,
                        )
                        nc.tensor.matmul(
                            ps1[:], lhsT=lhs, rhs=wqk[:, 512:1024],
                            start=first, stop=last,
                        )
                    evict(ps0, ps1, nb * S_BLK + s_lo)

    nc.compile()
    return nc


def _get_nc():
    if "nc" not in _nc_cache:
        _nc_cache["nc"] = _build_kernel()
    return _nc_cache["nc"]


def _get_nc_reduce():
    if "nc_red" not in _nc_cache:
        _nc_cache["nc_red"] = _build_reduce_kernel()
    return _nc_cache["nc_red"]


def _shard_inputs(x, W, spart=None):
    x2 = np.asarray(x, dtype=np.float32).reshape(S_TOT, I_DIM)
    W2 = np.ascontiguousarray(np.asarray(W, dtype=np.float32))

    # [nb, p, ko, s']: x5[nb, p, ko, s'] = x_half[nb*S_BLK+s', ko*P+p]
    x5_slices = [
        np.ascontiguousarray(
            x2[r * S_CORE : (r + 1) * S_CORE, :]
            .reshape(N_SBLK, S_BLK, KP, P)
            .transpose(0, 3, 2, 1)
        )
        for r in range(R_CORES)
    ]
    wT_slices = [
        np.ascontiguousarray(W2[c * O_CORE : (c + 1) * O_CORE, :].T)
        for c in range(C_CORES)
    ]
    if GATHER == "cc":
        wred_slices = [
            np.ascontiguousarray(W2[c * W_RED : (c + 1) * W_RED, :].T)
            for c in range(N_CORES)
        ]
    in_maps = []
    for core in range(N_CORES):
        ri, ci = core // C_CORES, core % C_CORES
        m = {"x5": x5_slices[ri], "wT": wT_slices[ci]}
        if GATHER == "cc":
            m["wred"] = wred_slices[core]
        elif spart is not None:
            m["spart"] = spart
        in_maps.append(m)
    return in_maps


def _gather_output(results):
    y = np.empty((S_TOT, O_DIM), dtype=np.float32)
    for core in range(N_CORES):
        ri, ci = core // C_CORES, core % C_CORES
        y[ri * S_CORE : (ri + 1) * S_CORE, ci * O_CORE : (ci + 1) * O_CORE] = (
            results[core]["y"]
        )
    return y.reshape(B, SEQ, O_DIM)


def _prime_axon_profile():
    """Refresh the axon profile side-channel: one tiny device execute plus a
    start/stop pair. `axon_start_nrt_profile` returns -1 unless the client has
    been active recently, so this runs right before the profiled execute."""
    try:
        import ctypes
        import tempfile

        import jax
        import jax.numpy as jnp

        np.asarray(jax.jit(lambda a: a + 1)(jnp.zeros((8,))))
        lib = ctypes.CDLL("/opt/axon/libaxon_pjrt.so")
        lib.axon_start_nrt_profile.argtypes = [
            ctypes.POINTER(ctypes.c_int64),
            ctypes.c_size_t,
        ]
        lib.axon_start_nrt_profile.restype = ctypes.c_int64
        lib.axon_stop_nrt_profile.argtypes = [ctypes.c_char_p]
        lib.axon_stop_nrt_profile.restype = ctypes.c_int64
        ids = (ctypes.c_int64 * 1)(0)
        rc = lib.axon_start_nrt_profile(ids, 1)
        if rc == 0:
            lib.axon_stop_nrt_profile(tempfile.mkdtemp().encode())
        print(f"axon profile primed (rc={rc})")
    except Exception as e:
        print(f"axon profile priming failed: {type(e).__name__}: {e}")


def _run_reduce(W, **spmd_kwargs):
    """Pass 1: per-core |W|-slice partials. Host only concatenates."""
    from concourse.bass_utils import run_bass_kernel_spmd

    nc1 = _get_nc_reduce()
    W2 = np.ascontiguousarray(np.asarray(W, dtype=np.float32))
    in_maps = [
        {"wredr": np.ascontiguousarray(W2[c * W_RED : (c + 1) * W_RED, :])}
        for c in range(N_CORES)
    ]
    res = run_bass_kernel_spmd(
        nc1, in_maps, core_ids=list(range(N_CORES)), **spmd_kwargs
    )
    spart = np.ascontiguousarray(
        np.concatenate([res.results[c]["spart"] for c in range(N_CORES)], axis=1)
    )  # [128, 8]
    return spart, res


def _run(x, W, **spmd_kwargs):
    import time

    from concourse.bass_utils import run_bass_kernel_spmd

    nc = _get_nc()
    last_err = None
    for attempt in range(3):
        _prime_axon_profile()
        try:
            res1 = None
            spart = None
            if GATHER == "twopass":
                spart, res1 = _run_reduce(W, **spmd_kwargs)
            in_maps = _shard_inputs(x, W, spart=spart)
            res = run_bass_kernel_spmd(
                nc, in_maps, core_ids=list(range(N_CORES)), **spmd_kwargs
            )
            res.reduce_pass = res1
            return _gather_output(res.results), res
        except Exception as e:  # transient device wedges recover on retry
            last_err = e
            time.sleep(5.0 * (attempt + 1))
    raise last_err


def kernel(x, W):
    out, _ = _run(x, W)
    return out


# revision 13
# speedup vs baseline: 1.2717x; 1.0228x over previous
"""BitLinear (ternary-quantized linear) kernel for Trainium2, 8 NeuronCores.

Reference computation:
    scale = mean(|W|);  Wq = round(W / (scale + 1e-5));  y = (x @ Wq^T) * scale

Distribution (2x4 grid over 8 cores):
  - batch/sequence dim (8192 rows of x) split 2 ways  -> ri = core // 4
  - out_features dim (4096 rows of W) split 4 ways    -> ci = core % 4
  Each core computes y block [4096 s, 1024 o].

The global mean(|W|) must be EXACT: quantizing with a per-core quarter
mean measures rel_err 2.67e-2 on the harness inputs (~3.6k of 16.7M
weights flip their rounding boundary) vs the 2e-2 gate. Exact-scale
bf16 measures 1.72e-3.

Scale strategies (BITLIN_GATHER):
  "twopass" (default): two NEFF executions.
      Pass 1 (~40us): each core reduces a distinct 1/8 of W (row-major
      slice -> 16KB/partition DMA rows at line rate) to a [128,1]
      fp32 partial. The host only CONCATENATES the 8 partials (layout,
      no FLOPs) and feeds the [128,8] block to every core.
      Pass 2 (~570us): reduce the replicated partials on-device (~2us),
      broadcast via ones-matmul (full-precision fp32 2-pass PE mode),
      then stream wT once: quantize on arrival (DVE/ACT parity split),
      matmuls from ~20us in wq arrival order.
  "cc": single kernel with an ncfw AllGather combining the partials.
      The ncfw path has a hard ~110us scale floor on this stack (TOPSP
      wakeup barrier ~50us starting at ~21us + trigger + ~26us Mesh
      AllGather for 512B): 671us total measured.
  Abandoned: peer-SBUF SDMA gather deadlocks on HW (core launch skew
  exceeds the send offset; fixed-threshold semaphore handshakes lose
  increments and wedge the exec unit). Dummy warm-up AllGather: ncfw
  serializes collectives on one stream, pushing the real one later
  (697us measured).

Host side does layout only: x^T / W^T slices (the contraction dim i must
be the SBUF partition dim on both matmul operands), the pass-1 partial
concat, and stitching output blocks. All FLOPs (reduction, quantization,
matmul, rescale) run on device.

The matmul phase runs at the power-capped PE roofline: the board GPIO
throttler holds the PE at K=13/16 of 2.4GHz under sustained 8-core load,
so 34.4 GFLOP/core needs ~538us of PE time (measured 534us, >97%
occupancy in the steady loop).
"""

import os
import sys
import types

import numpy as np


def _ensure_axon_hooks_module():
    """Some images lack ``antenv.axon_hooks``; ``run_bass_kernel_spmd`` imports
    it unconditionally when tracing is requested. Install a no-op fallback so a
    BASS_TRACE=1 environment degrades to "no trace" instead of crashing."""
    try:
        import antenv.axon_hooks  # noqa: F401
        return
    except ImportError:
        pass
    try:
        import antenv
    except ImportError:
        return
    mod = types.ModuleType("antenv.axon_hooks")
    mod._hook = None

    def set_axon_ntff_profile_hook(h):
        mod._hook = h

    def get_axon_ntff_profile_hook():
        return mod._hook

    mod.set_axon_ntff_profile_hook = set_axon_ntff_profile_hook
    mod.get_axon_ntff_profile_hook = get_axon_ntff_profile_hook
    sys.modules["antenv.axon_hooks"] = mod
    antenv.axon_hooks = mod


_ensure_axon_hooks_module()

# ---- problem constants (hardcoded per contract) ----
B, SEQ, I_DIM, O_DIM = 4, 2048, 4096, 4096
S_TOT = B * SEQ            # 8192
R_CORES, C_CORES = 2, 4    # grid: batch x out_features
N_CORES = R_CORES * C_CORES
S_CORE = S_TOT // R_CORES  # 4096 sequence rows per core
O_CORE = O_DIM // C_CORES  # 1024 output features per core
P = 128
KP = I_DIM // P            # 32 contraction chunks
S_BLK = 256                # s columns per x load block
N_SBLK = S_CORE // S_BLK   # 16
W_RED = O_DIM // N_CORES   # 512: rows of W reduced per core for mean|W|
RT = W_RED // P            # 4 reduction chunks in pass 1
WCH = 2                    # ko chunks per W staging tile ([128, 2, 1024] = 1 MB)
N_WT = KP // WCH           # 16 stage/quantize tiles
MAGIC = 1.5 * (2.0 ** 23)  # fp32 round-to-nearest-even trick constant
EPS = 1e-5
GATHER = os.environ.get("BITLIN_GATHER", "twopass")

_nc_cache = {}


def _build_reduce_kernel():
    """Pass 1: spart[p] = sum over its 1/8 slice of |W| (per-partition)."""
    import concourse.mybir as mybir
    import concourse.tile as tile
    from concourse import bacc

    f32 = mybir.dt.float32
    Alu = mybir.AluOpType
    Act = mybir.ActivationFunctionType

    nc = bacc.Bacc(
        "TRN2",
        target_bir_lowering=False,
        debug=False,
        enable_asserts=False,
        num_devices=N_CORES,
    )
    # row-major [512, 4096] slice: 16 KB contiguous per (partition, chunk)
    wredr = nc.dram_tensor("wredr", [W_RED, I_DIM], f32, kind="ExternalInput")
    spart = nc.dram_tensor("spart", [P, 1], f32, kind="ExternalOutput")
    wr = wredr.ap().rearrange("(t p) i -> p t i", p=P)  # [128, 4, 4096]

    with tile.TileContext(nc) as tc:
        with (
            tc.tile_pool(name="wst", bufs=4) as wst,
            tc.tile_pool(name="st", bufs=1) as st,
        ):
            part = st.tile([P, RT], f32)
            for t in range(RT):
                wt = wst.tile([P, 1, I_DIM], f32, tag="w")
                nc.sync.dma_start(wt[:], wr[:, t : t + 1, :])
                if t % 2 == 0:
                    nc.vector.tensor_reduce(
                        part[:, t : t + 1],
                        wt[:],
                        axis=mybir.AxisListType.XY,
                        op=Alu.add,
                        apply_absolute_value=True,
                    )
                else:
                    nc.scalar.activation(
                        wt[:], wt[:], Act.Abs, accum_out=part[:, t : t + 1]
                    )
            accv = st.tile([P, 1], f32)
            nc.vector.tensor_reduce(
                accv[:], part[:], axis=mybir.AxisListType.X, op=Alu.add
            )
            nc.sync.dma_start(spart.ap()[:, :], accv[:])

    nc.compile()
    return nc


def _build_kernel():
    import concourse.mybir as mybir
    import concourse.tile as tile
    from concourse import bacc
    from concourse.tile import add_dep_helper

    f32 = mybir.dt.float32
    bf16 = mybir.dt.bfloat16
    Alu = mybir.AluOpType
    Act = mybir.ActivationFunctionType

    nc = bacc.Bacc(
        "TRN2",
        target_bir_lowering=False,
        debug=False,
        enable_asserts=False,
        num_devices=N_CORES,
    )

    twopass = GATHER == "twopass"
    # x arrives block-contiguous: [nb, p, ko, s'] so one x block is 128
    # partition-contiguous 32 KB runs. The naive [I, S] column-slice layout
    # fragments each 4 MiB block into 4096x1KB descriptors — measured ~60us
    # per block under wT contention, which stalled the first matmul to ~72us.
    x5 = nc.dram_tensor("x5", [N_SBLK, P, KP, S_BLK], f32, kind="ExternalInput")
    wT = nc.dram_tensor("wT", [I_DIM, O_CORE], f32, kind="ExternalInput")
    if twopass:
        spart = nc.dram_tensor("spart", [P, N_CORES], f32, kind="ExternalInput")
    else:
        wred = nc.dram_tensor("wred", [I_DIM, W_RED], f32, kind="ExternalInput")
        wred_r = wred.ap().rearrange("(ko p) o -> p ko o", p=P)  # [128, 32, 512]
    y = nc.dram_tensor("y", [S_CORE, O_CORE], f32, kind="ExternalOutput")

    x5_ap = x5.ap()                                        # [16, 128, 32, 256]
    wT_r = wT.ap().rearrange("(ko p) o -> p ko o", p=P)    # [128, 32, 1024]
    y_ap = y.ap()

    with tile.TileContext(nc) as tc:
        with (
            tc.tile_pool(name="const", bufs=1) as const_pool,
            tc.tile_pool(name="stats", bufs=1) as stats,
            tc.tile_pool(name="wstage", bufs=7) as wstage,
            tc.tile_pool(name="wq", bufs=1) as wq_pool,
            tc.tile_pool(name="xbf", bufs=3) as xbf_pool,
            tc.tile_pool(name="yout", bufs=3) as yout_pool,
            tc.tile_pool(name="psum_s", bufs=1, space="PSUM") as psum_s,
            tc.tile_pool(name="psum_mm", bufs=3, space="PSUM") as psum_mm,
            tc.tile_pool(name="dram", bufs=1, space="DRAM") as dram_pool,
        ):
            wq_tiles = [
                wq_pool.tile([P, WCH, O_CORE], bf16, tag=f"wq{t}", name=f"wq{t}")
                for t in range(N_WT)
            ]

            # ---------- Phase A/B: per-core |W| partials -> global sum ----------
            bounce_dma = None
            gate = None
            if twopass:
                # partials were computed in pass 1; every core got the same
                # [128, 8] block. ~3us DMA + ~1us reduce.
                spart_sb = stats.tile([P, N_CORES], f32)
                nc.sync.dma_start(spart_sb[:], spart.ap())
                acc_r = stats.tile([P, 1], f32)
                nc.vector.tensor_reduce(
                    acc_r[:], spart_sb[:], axis=mybir.AxisListType.X, op=Alu.add
                )
            else:
                # single-kernel path: reduce the wred slice here, AllGather.
                n_rtiles = KP // 4  # 8 tiles [128, 4, 512] = 1 MB each
                red_all = stats.tile([P, n_rtiles], f32)
                for t in range(n_rtiles):
                    wt = wstage.tile([P, 4, W_RED], f32, tag="wstage")
                    nc.sync.dma_start(wt[:], wred_r[:, t * 4 : (t + 1) * 4, :])
                    if t % 2 == 0:
                        nc.vector.tensor_reduce(
                            red_all[:, t : t + 1],
                            wt[:],
                            axis=mybir.AxisListType.XY,
                            op=Alu.add,
                            apply_absolute_value=True,
                        )
                    else:
                        nc.scalar.activation(
                            wt[:], wt[:], Act.Abs, accum_out=red_all[:, t : t + 1]
                        )
                acc = stats.tile([P, 1], f32)
                nc.vector.tensor_reduce(
                    acc[:], red_all[:], axis=mybir.AxisListType.X, op=Alu.add
                )
                cc_in = dram_pool.tile([P, 1], f32)
                cc_out = dram_pool.tile([N_CORES * P, 1], f32, addr_space="Shared")
                bounce_dma = nc.sync.dma_start(cc_in[:], acc[:])
                gate = nc.gpsimd.collective_compute(
                    "AllGather",
                    Alu.bypass,
                    replica_groups=[list(range(N_CORES))],
                    ins=[cc_in.opt()],
                    outs=[cc_out.opt()],
                )
                # read back as [128, 8]: partition p, free r <- dram[r*128 + p].
                # Keep this per-partition tree reduction: a flat 1024-element
                # sequential sum lands measurably further from the reference's
                # fp32 summation.
                acc_g = stats.tile([P, N_CORES], f32)
                nc.sync.dma_start(
                    acc_g[:], cc_out.rearrange("(r p) one -> p (r one)", p=P)
                )
                acc_r = stats.tile([P, 1], f32)
                nc.vector.tensor_reduce(
                    acc_r[:], acc_g[:], axis=mybir.AxisListType.X, op=Alu.add
                )

            # ---------- Phase C: scale scalars, broadcast to all partitions ----------
            # global sum broadcast: ones^T @ acc_r -> every partition = full
            # sum (the framework emits the exact 2-pass fp32 PE mode here)
            inv_numel = 1.0 / (float(I_DIM) * float(O_DIM))
            ones_b = const_pool.tile([P, P], f32)
            nc.vector.memset(ones_b[:], 1.0)
            ps_b = psum_s.tile([P, 1], f32)
            nc.tensor.matmul(ps_b[:], lhsT=ones_b[:], rhs=acc_r[:], start=True, stop=True)

            # sinv first: it gates quantization (scale_t is only needed at
            # output eviction, much later)
            seps_t = stats.tile([P, 1], f32)   # scale + eps
            nc.vector.tensor_scalar(
                seps_t[:], ps_b[:], inv_numel, EPS, op0=Alu.mult, op1=Alu.add
            )
            sinv_t = stats.tile([P, 1], f32)   # 1 / (scale + eps)
            nc.vector.reciprocal(sinv_t[:], seps_t[:])
            scale_t = stats.tile([P, 1], f32)  # mean(|W|)
            nc.vector.tensor_scalar_mul(scale_t[:], ps_b[:], inv_numel)

            # ---------- Phase D: quantize W -> bf16 integers (DVE + ACT split) ----------
            # single pass over wT, staged in plain order on the Sync HWDGE
            # queue only (issuing DMAs from nc.scalar serializes the triggers
            # behind the ACT compute chain on the same engine FIFO). Parity
            # split across DVE/ACT so both engines chew arrivals concurrently.
            first_done = False
            wt_dmas = []
            for t in range(N_WT):
                wt = wstage.tile([P, WCH, O_CORE], f32, tag="wstage")
                dma = nc.sync.dma_start(wt[:], wT_r[:, t * WCH : (t + 1) * WCH, :])
                wt_dmas.append(dma)
                if not first_done and bounce_dma is not None:
                    first_done = True
                    # cc path: keep wred DMAs exclusive on the queue until the
                    # collective input is on its way
                    add_dep_helper(dma.ins, bounce_dma.ins, sync=False,
                                   reason="stage wT after AR input bounce")
                if t % 2 == 0:
                    # wn = W * (1/(scale+eps)) + MAGIC  (fp32, in place)
                    nc.vector.tensor_scalar(
                        wt[:], wt[:], sinv_t[:], MAGIC, op0=Alu.mult, op1=Alu.add
                    )
                    # wq = (wn - MAGIC) cast to bf16  (exact small integers)
                    nc.vector.tensor_scalar_sub(wq_tiles[t][:], wt[:], MAGIC)
                else:
                    nc.scalar.activation(
                        wt[:], wt[:], Act.Copy, bias=MAGIC, scale=sinv_t[:]
                    )
                    nc.scalar.activation(
                        wq_tiles[t][:], wt[:], Act.Copy, bias=-MAGIC, scale=1.0
                    )

            # ---------- Phase E: y = (x @ Wq^T) * scale ----------
            def evict(ps0, ps1, row):
                yo = yout_pool.tile([P, O_CORE], f32, name="yo")
                nc.vector.tensor_scalar_mul(yo[:, 0:512], ps0[:], scale_t[:])
                nc.vector.tensor_scalar_mul(yo[:, 512:1024], ps1[:], scale_t[:])
                nc.sync.dma_start(y_ap[row : row + P, :], yo[:])

            x_blocks = []
            for nb in range(3):
                xb = xbf_pool.tile([P, KP, S_BLK], bf16, tag="xb", name=f"xb{nb}")
                # SWDGE casts fp32 -> bf16 inline during the HBM->SBUF DMA
                xdma = nc.gpsimd.dma_start(xb[:], x5_ap[nb])
                if gate is not None:
                    # don't let x descriptor-gen delay the gather trigger on
                    # the gpsimd queue
                    add_dep_helper(xdma.ins, gate.ins, sync=False,
                                   reason="x load after gather trigger")
                elif nb > 0:
                    # x block 0 overlaps the wT stream (the fast path gates on
                    # it ~10us after the scale); blocks 1-2 wait for most of
                    # the quantization-gating wT stream to LAND (real sem dep:
                    # an issue-order dep would still let their SDMA packets
                    # round-robin against wT and halve its bandwidth — that
                    # pushed the first fast-path matmul from ~20us to ~67us)
                    add_dep_helper(xdma.ins, wt_dmas[2 if nb == 1 else 8].ins,
                                   sync=True,
                                   reason="x prefetch behind wT stream")
                x_blocks.append(xb)

            # Fast path: s-tiles 0,1 (x block 0) accumulate in 4 PSUM banks,
            # consuming wq tiles in staging order so the PE starts on the first
            # quantized tile. Gating any early matmul on later x blocks stalls
            # the in-order PE queue at t=0, so s-tile 2 (x block 1) only joins
            # from tile 8 (~37us, x1 resident by ~30us); its missing k-chunks
            # 0..15 run as a PE-bound tail from the then-resident wq tiles.
            # The extra width matters twice: it fills the arrival-limited idle,
            # and it keeps PE gaps under the ~3.4us HAM window that would
            # re-throttle the PE clock to 1.2GHz.
            fast_units = []  # (psum, s_tile, o_half, join_tile)
            for stg in range(2):
                ps0 = psum_mm.tile([P, 512], f32, tag="mm0", name=f"fps0_{stg}")
                ps1 = psum_mm.tile([P, 512], f32, tag="mm1", name=f"fps1_{stg}")
                fast_units.append((ps0, stg, 0, 0))
                fast_units.append((ps1, stg, 1, 0))
            JOIN = N_WT // 2
            ps0_s2 = psum_mm.tile([P, 512], f32, tag="mm0", name="fps0_2")
            ps1_s2 = psum_mm.tile([P, 512], f32, tag="mm1", name="fps1_2")
            fast_units.append((ps0_s2, 2, 0, JOIN))
            fast_units.append((ps1_s2, 2, 1, JOIN))

            def fast_mm(ps, stg, half, t, start, stop):
                xb = x_blocks[stg // 2]
                s_lo = (stg % 2) * P
                for kk in range(WCH):
                    nc.tensor.matmul(
                        ps[:],
                        lhsT=xb[:, t * WCH + kk, s_lo : s_lo + P],
                        rhs=wq_tiles[t][:, kk, 512 * half : 512 * (half + 1)],
                        start=start and kk == 0,
                        stop=stop and kk == WCH - 1,
                    )

            for t in range(N_WT):
                for ps, stg, half, join in fast_units:
                    if t < join:
                        continue
                    fast_mm(ps, stg, half, t,
                            start=(t == join),
                            stop=(join == 0 and t == N_WT - 1))
            for t in range(JOIN):  # s-tile 2 tail: k-chunks 0..15
                for ps, stg, half, join in fast_units:
                    if join == 0:
                        continue
                    fast_mm(ps, stg, half, t, start=False, stop=(t == JOIN - 1))
            for stg in range(2):
                evict(fast_units[2 * stg][0], fast_units[2 * stg + 1][0], stg * P)
            evict(ps0_s2, ps1_s2, 2 * P)

            # Steady state (s-tiles 3..31)
            for nb in range(1, N_SBLK):
                if nb >= 3:
                    xb = xbf_pool.tile([P, KP, S_BLK], bf16, tag="xb", name=f"xb{nb}")
                    nc.gpsimd.dma_start(xb[:], x5_ap[nb])
                else:
                    xb = x_blocks[nb]
                for st in range(S_BLK // P):
                    if nb == 1 and st == 0:
                        continue  # s-tile 2 covered by the fast path
                    ps0 = psum_mm.tile([P, 512], f32, tag="mm0", name="ps0")
                    ps1 = psum_mm.tile([P, 512], f32, tag="mm1", name="ps1")
                    s_lo = st * P
                    for k in range(KP):
                        lhs = xb[:, k, s_lo : s_lo + P]
                        wqk = wq_tiles[k // WCH][:, k % WCH, :]
                        first, last = (k == 0), (k == KP - 1)
                        nc.tensor.matmul(
                            ps0[:], lhsT=lhs, rhs=wqk[:, 0:512],
                            start=first, stop=last,
                        )
                        nc.tensor.matmul(
                            ps1[:], lhsT=lhs, rhs=wqk[:, 512:1024],
                            start=first, stop=last,
                        )
                    evict(ps0, ps1, nb * S_BLK + s_lo)

    nc.compile()
    return nc


def _get_nc():
    if "nc" not in _nc_cache:
        _nc_cache["nc"] = _build_kernel()
    return _nc_cache["nc"]


def _get_nc_reduce():
    if "nc_red" not in _nc_cache:
        _nc_cache["nc_red"] = _build_reduce_kernel()
    return _nc_cache["nc_red"]


def _shard_inputs(x, W, spart=None):
    x2 = np.asarray(x, dtype=np.float32).reshape(S_TOT, I_DIM)
    W2 = np.ascontiguousarray(np.asarray(W, dtype=np.float32))

    # [nb, p, ko, s']: x5[nb, p, ko, s'] = x_half[nb*S_BLK+s', ko*P+p]
    x5_slices = [
        np.ascontiguousarray(
            x2[r * S_CORE : (r + 1) * S_CORE, :]
            .reshape(N_SBLK, S_BLK, KP, P)
            .transpose(0, 3, 2, 1)
        )
        for r in range(R_CORES)
    ]
    wT_slices = [
        np.ascontiguousarray(W2[c * O_CORE : (c + 1) * O_CORE, :].T)
        for c in range(C_CORES)
    ]
    if GATHER == "cc":
        wred_slices = [
            np.ascontiguousarray(W2[c * W_RED : (c + 1) * W_RED, :].T)
            for c in range(N_CORES)
        ]
    in_maps = []
    for core in range(N_CORES):
        ri, ci = core // C_CORES, core % C_CORES
        m = {"x5": x5_slices[ri], "wT": wT_slices[ci]}
        if GATHER == "cc":
            m["wred"] = wred_slices[core]
        elif spart is not None:
            m["spart"] = spart
        in_maps.append(m)
    return in_maps


def _gather_output(results):
    y = np.empty((S_TOT, O_DIM), dtype=np.float32)
    for core in range(N_CORES):
        ri, ci = core // C_CORES, core % C_CORES
        y[ri * S_CORE : (ri + 1) * S_CORE, ci * O_CORE : (ci + 1) * O_CORE] = (
            results[core]["y"]
        )
    return y.reshape(B, SEQ, O_DIM)


def _prime_axon_profile():
    """Refresh the axon profile side-channel: one tiny device execute plus a
    start/stop pair. `axon_start_nrt_profile` returns -1 unless the client has
    been active recently, so this runs right before the profiled execute."""
    try:
        import ctypes
        import tempfile

        import jax
        import jax.numpy as jnp

        np.asarray(jax.jit(lambda a: a + 1)(jnp.zeros((8,))))
        lib = ctypes.CDLL("/opt/axon/libaxon_pjrt.so")
        lib.axon_start_nrt_profile.argtypes = [
            ctypes.POINTER(ctypes.c_int64),
            ctypes.c_size_t,
        ]
        lib.axon_start_nrt_profile.restype = ctypes.c_int64
        lib.axon_stop_nrt_profile.argtypes = [ctypes.c_char_p]
        lib.axon_stop_nrt_profile.restype = ctypes.c_int64
        ids = (ctypes.c_int64 * 1)(0)
        rc = lib.axon_start_nrt_profile(ids, 1)
        if rc == 0:
            lib.axon_stop_nrt_profile(tempfile.mkdtemp().encode())
        print(f"axon profile primed (rc={rc})")
    except Exception as e:
        print(f"axon profile priming failed: {type(e).__name__}: {e}")


def _run_reduce(W, **spmd_kwargs):
    """Pass 1: per-core |W|-slice partials. Host only concatenates."""
    from concourse.bass_utils import run_bass_kernel_spmd

    nc1 = _get_nc_reduce()
    W2 = np.ascontiguousarray(np.asarray(W, dtype=np.float32))
    in_maps = [
        {"wredr": np.ascontiguousarray(W2[c * W_RED : (c + 1) * W_RED, :])}
        for c in range(N_CORES)
    ]
    res = run_bass_kernel_spmd(
        nc1, in_maps, core_ids=list(range(N_CORES)), **spmd_kwargs
    )
    spart = np.ascontiguousarray(
        np.concatenate([res.results[c]["spart"] for c in range(N_CORES)], axis=1)
    )  # [128, 8]
    return spart, res


def _run(x, W, **spmd_kwargs):
    import time

    from concourse.bass_utils import run_bass_kernel_spmd

    nc = _get_nc()
    last_err = None
    for attempt in range(3):
        _prime_axon_profile()
        try:
            res1 = None
            spart = None
            if GATHER == "twopass":
                spart, res1 = _run_reduce(W, **spmd_kwargs)
            in_maps = _shard_inputs(x, W, spart=spart)
            res = run_bass_kernel_spmd(
                nc, in_maps, core_ids=list(range(N_CORES)), **spmd_kwargs
            )
            res.reduce_pass = res1
            return _gather_output(res.results), res
        except Exception as e:  # transient device wedges recover on retry
            last_err = e
            time.sleep(5.0 * (attempt + 1))
    raise last_err


def kernel(x, W):
    out, _ = _run(x, W)
    return out
